# revision 28
# baseline (speedup 1.0000x reference)
"""nn_CoMet Trainium2 kernel.

Math (per batch element s in R^16):
  MLP: h1 = logsig(s@W1.T); h2 = h1 + logsig(h1@W2.T); h3 = h2 + logsig(h2@W3.T)
       nnout = h3@W4.T ; d = nnout[:16]; com-rows C = W4[16:20]
  J = d com/d s  (reverse mode through the MLP, 4 rows)
  out = d - J^T (J J^T)^{-1} J d      (== the QR-based projection in the reference)

Kernel design (per core, batch 32768, hidden-major layout [hidden on partitions,
elements on free axis], tiles of 512 elements):
  - forward matmuls in float32r (full-rate fp32-ish PE mode), activations via the
    natural_log_exp table set only:  e=Exp(-a), Lg=Ln(e+1) (= -logsig(a)),
    r=Exp(-Lg), s=e*r (= sig(-a)).  hm_k = -h_k accumulates Lg terms; signs are
    folded into the (host-negated) weights.
  - backward: V accumulates IN PSUM: t3 = s3 @ diag(C_i)W3, then U = (V2*s2)@W2
    accumulated into the same bank; the "+C_i" term is applied on the fly by
    scalar_tensor_tensor (per-partition scalar) when reading PSUM.
  - J and d are produced already transposed (element-major) by using the batch
    tile as the *stationary* matmul operand, so the 4x4 normal-equation solve
    runs on [128 elems, groups, comps] tiles with full lane utilization.
"""

import numpy as np
import ml_dtypes

import concourse.bass as bass
import concourse.mybir as mybir
import concourse.tile as tile

NCORES = 8
B = 262144
BP = B // NCORES          # 32768 per core
NS = 16
NH = 256
NCOM = 4
TN = 512                  # elements per pipeline tile
NTILES = BP // TN         # 64
SUP = 2                   # tiles per supertile (transpose/postproc unit) -> 1024 elems
GRPS = SUP * TN // 128    # 8 groups of 128 elems per supertile
SOLVE_SUPS = 8            # supertiles per batched solve -> 8192 elems
SROWS = SOLVE_SUPS * GRPS # 64

F32 = mybir.dt.float32
F32R = mybir.dt.float32r
BF16 = mybir.dt.bfloat16
FP8 = mybir.dt.float8e4
AF = mybir.ActivationFunctionType
OP = mybir.AluOpType
AX = mybir.AxisListType
DRM = mybir.MatmulPerfMode.DoubleRow

_cache = {}


# ---------------------------------------------------------------- drain patch
def _patch_tile_drain():
    """walrus in this container rejects >1 sem wait on the TileContext final
    drain ("Too many sync wait commands"); split the waits across several
    drain instructions (1 wait each)."""
    if getattr(tile.TileContext, "_comet_patched", False):
        return
    from concourse.vector_clock import ScopedClock

    def _drain_and_barrier(self, tick_clock, wait_clock):
        nc = self.nc
        drain_inst = nc.sync.drain()
        wait_clock.add_sem_waits(
            drain_inst.ins, ScopedClock({None: tick_clock.global_clock})
        )
        si = drain_inst.ins.sync_info
        waits = list(si.on_wait) if si is not None and si.on_wait else []
        if len(waits) > 1:
            si.on_wait = waits[:1]
            for w in waits[1:]:
                extra = nc.sync.drain()
                esi = extra.ins.sync_info
                if esi is None:
                    import bass_rust
                    extra.ins.sync_info = bass_rust.SyncInfo(
                        on_wait=[w], on_update=[]
                    )
                else:
                    esi.on_wait = list(esi.on_wait or []) + [w]
        nc.all_engine_barrier()
        assert self.sems is not None
        popped = nc._tile_sem_poison_stack.pop()
        assert popped is self._sem_poison
        nc.clear_and_free_semaphores(list(self.sems.allocated().values()))
        nc.all_engine_barrier()

    tile.TileContext._drain_and_barrier = _drain_and_barrier
    tile.TileContext._comet_patched = True


def _split_multi_waits(nc):
    """This container's walrus rejects instructions with more than one sync
    wait command.  Hoist extra waits onto injected same-engine NOPs placed
    immediately before the offending instruction."""
    import bass_rust

    for f in nc.m.functions:
        for b in f.blocks:
            insts = list(b.instructions)
            out, dirty = [], False
            for inst in insts:
                si = inst.sync_info
                waits = list(si.on_wait) if si is not None and si.on_wait else []
                if len(waits) > 1:
                    dirty = True
                    for k, wx in enumerate(waits[:-1]):
                        nop = mybir.InstNoOp(name=f"{inst.name}-ws{k}")
                        nop.engine = inst.engine
                        nop.sync_info = bass_rust.SyncInfo(
                            on_wait=[wx], on_update=[])
                        out.append(nop)
                    si.on_wait = waits[-1:]
                out.append(inst)
            if dirty:
                b.instructions = out


# ------------------------------------------------------- scales (fp8 ranges)
ALPHA = 64.0              # stage_a products (M3, C rows) pre-scale: v_ps = a*V
KB = 8.0                  # v2s2 = (v_ps * KB/ALPHA) . s2 = KB*(V.s2)  [fp8]
SW2 = ALPHA / KB          # W2 fp8 scale so stage_c restores factor ALPHA


# ---------------------------------------------------------------- host prep
def _prep_weights(W1, W2, W3, W4):
    W1 = np.asarray(W1, np.float32)
    W2 = np.asarray(W2, np.float32)
    W3 = np.asarray(W3, np.float32)
    W4 = np.asarray(W4, np.float32)
    W4d, C = W4[:NS], W4[NS:]
    bf = ml_dtypes.bfloat16
    f8 = mybir.dt.np(mybir.dt.float8e4)

    def khalf(a, m):   # [256, m] -> [128, 2, m]
        return np.ascontiguousarray(a.reshape(2, 128, m).transpose(1, 0, 2))

    def hilo(a):  # fp8 hi + residual lo split: dec(hi)+dec(lo) ~ a to ~2^-8
        hi = a.astype(f8)
        lo = (a - hi.astype(np.float32)).astype(f8)
        return hi, lo

    L1 = np.ascontiguousarray(W1.T)                       # [16,256] lhsT for a1
    L2 = khalf(W2.T, NH)                                  # [128,2,256]
    L3 = khalf(W3.T, NH)
    L4 = khalf(W4d.T, NS)                                 # [128,2,16]
    M3 = np.stack([C[i][:, None] * W3 for i in range(4)]) # [4,256,256]
    M3 = np.ascontiguousarray(
        (M3 * ALPHA).reshape(4, 2, 128, NH).transpose(2, 1, 0, 3)  # [128,2,4,256]
    ).astype(np.float32)
    W2hi, W2lo = hilo(khalf(W2 * SW2, NH))                # [128,2,256] DR lhsT for U
    W1b = khalf(W1, NS).astype(bf)                        # [128,2,16] rhs for Jt
    # C rows as a K=1(x2 hi/lo) DR lhsT: [1, 2, 4i, 256] fp8, ALPHA-scaled
    Chi, Clo = hilo(np.ascontiguousarray(
        C[None, :, :] * ALPHA))                           # [1,4,256] each
    Cb8 = np.ascontiguousarray(np.stack([Chi, Clo], axis=1))  # [1,2,4,256]
    return {
        "L1": L1, "L2": L2, "L3": L3, "L4": L4,
        "M3": M3, "W2hi": W2hi, "W2lo": W2lo, "W1b": W1b, "Cb8": Cb8,
    }


# ---------------------------------------------------------------- build
def _build(ntiles=NTILES, a_bufs=1, v_bufs=2):
    _patch_tile_drain()
    nc = bass.Bass()

    zT = nc.dram_tensor("zT", [NS, BP], F32R, kind="ExternalInput")
    dL1 = nc.dram_tensor("L1", [NS, NH], F32R, kind="ExternalInput")
    dL2 = nc.dram_tensor("L2", [128, 2, NH], F32R, kind="ExternalInput")
    dL3 = nc.dram_tensor("L3", [128, 2, NH], F32R, kind="ExternalInput")
    dL4 = nc.dram_tensor("L4", [128, 2, NS], F32R, kind="ExternalInput")
    dM3 = nc.dram_tensor("M3", [128, 2, 4, NH], F32R, kind="ExternalInput")
    dW2hi = nc.dram_tensor("W2hi", [128, 2, NH], FP8, kind="ExternalInput")
    dW2lo = nc.dram_tensor("W2lo", [128, 2, NH], FP8, kind="ExternalInput")
    dW1b = nc.dram_tensor("W1b", [128, 2, NS], BF16, kind="ExternalInput")
    dCb8 = nc.dram_tensor("Cb8", [1, 2, 4, NH], FP8, kind="ExternalInput")
    out_d = nc.dram_tensor("out", [BP, NS], F32, kind="ExternalOutput")

    from contextlib import ExitStack
    with tile.TileContext(nc) as tc, ExitStack() as ctx:
        wpool = ctx.enter_context(tc.tile_pool(name="w", bufs=1))
        sb = ctx.enter_context(tc.tile_pool(name="sb", bufs=3))
        sb2 = ctx.enter_context(tc.tile_pool(name="sb2", bufs=3))
        sb2b = ctx.enter_context(tc.tile_pool(name="sb2b", bufs=2))
        mp1 = ctx.enter_context(tc.tile_pool(name="mp1", bufs=1))
        jpool = ctx.enter_context(tc.tile_pool(name="jp", bufs=2))
        mpool = ctx.enter_context(tc.tile_pool(name="mp", bufs=2))
        # Two PSUM rings chosen so every WAR wait lands on a dependency the
        # consumer already has:  psv (bufs=2) carries [a1, v0..v3] per tile —
        # a1's bank is freed by sig1 early, and each v-alloc's ring
        # predecessor died a full backward earlier.  psa (bufs=1) alternates
        # [a2, a3]: a2(t) waits sig3(t-1) (long done) and a3(t) waits sig2(t),
        # which gates a3's rhs (lg2) anyway.
        psv_pool = ctx.enter_context(tc.tile_pool(name="psv", bufs=2, space="PSUM"))
        psa_pool = ctx.enter_context(tc.tile_pool(name="psa", bufs=1, space="PSUM"))
        tps_pool = ctx.enter_context(tc.tile_pool(name="tps", bufs=1, space="PSUM"))

        # ---- load constants
        L1s = wpool.tile([NS, NH], F32R)
        nc.sync.dma_start(L1s[:], dL1[:])
        L2s = wpool.tile([128, 2, NH], F32R)
        nc.sync.dma_start(L2s[:], dL2[:])
        L3s = wpool.tile([128, 2, NH], F32R)
        nc.sync.dma_start(L3s[:], dL3[:])
        L4s = wpool.tile([128, 2, NS], F32R)
        nc.sync.dma_start(L4s[:], dL4[:])
        M3s = wpool.tile([128, 2, 4, NH], F32R)
        nc.sync.dma_start(M3s[:], dM3[:])
        W2his = wpool.tile([128, 2, NH], FP8)
        nc.sync.dma_start(W2his[:], dW2hi[:])
        W2los = wpool.tile([128, 2, NH], FP8)
        nc.sync.dma_start(W2los[:], dW2lo[:])
        W1s = wpool.tile([128, 2, NS], BF16)
        nc.sync.dma_start(W1s[:], dW1b[:])
        Cb8s = wpool.tile([1, 2, 4, NH], FP8)
        nc.sync.dma_start(Cb8s[:], dCb8[:])
        ones8 = wpool.tile([1, 2, TN], FP8)
        nc.vector.memset(ones8[:], 1.0)

        cur = {}       # current supertile PSUM accumulators
        grp = {}       # current solve-group SBUF state
        pending = []   # deferred postproc/solve emission closures

        # ------------------------------------------------ forward pieces
        def act_block(a_psum, layer, sdt=F32):
            # s = sigmoid(-a); h-term = logsig(a) = ln(1 - s)
            sg = sb2.tile([128, 2, TN], sdt, tag=f"s{layer}", name=f"s{layer}")
            nc.scalar.activation(sg[:], a_psum[:], AF.Sigmoid, scale=-1.0)
            lg = sb2.tile([128, 2, TN], F32R, tag=f"lg{layer}", name=f"lg{layer}")
            nc.scalar.activation(lg[:], sg[:], AF.Ln, bias=1.0, scale=-1.0)
            return lg, sg

        def fwd_part1(t):
            e0 = t * TN
            zt = sb2.tile([NS, TN], F32R, tag="zt")
            nc.sync.dma_start(zt[:], zT[:, e0:e0 + TN])
            a_ps = psv_pool.tile([128, 2, TN], F32, tag="psv", name="a1ps")
            for j in range(2):
                nc.tensor.matmul(a_ps[:, j], L1s[:, j * 128:(j + 1) * 128],
                                 zt[:], start=True, stop=True)
            hm1, s1b = act_block(a_ps, 1)   # hm1 == h1 = logsig(a1)
            return dict(hm1=hm1, s1b=s1b)

        def fwd_part2(t, F):
            a_ps = psa_pool.tile([128, 2, TN], F32, tag="psa", name="a2ps")
            for j in range(2):
                for k in range(2):
                    nc.tensor.matmul(a_ps[:, j],
                                     L2s[:, k, j * 128:(j + 1) * 128],
                                     F["hm1"][:, k], start=(k == 0), stop=(k == 1))
            lg2, s2b = act_block(a_ps, 2)
            hm2 = sb2.tile([128, 2, TN], F32R, tag="hm2")
            nc.gpsimd.tensor_tensor(hm2[:], F["hm1"][:], lg2[:], OP.add)
            F.update(hm2=hm2, s2b=s2b)

        def fwd_part3(t, F):
            a_ps = psa_pool.tile([128, 2, TN], F32, tag="psa", name="a3ps")
            for j in range(2):
                for k in range(2):
                    nc.tensor.matmul(a_ps[:, j],
                                     L3s[:, k, j * 128:(j + 1) * 128],
                                     F["hm2"][:, k], start=(k == 0), stop=(k == 1))
            lg3, s3b = act_block(a_ps, 3, sdt=F32R)
            hm3 = sb2.tile([128, 2, TN], F32R, tag="hm3")
            nc.gpsimd.tensor_tensor(hm3[:], F["hm2"][:], lg3[:], OP.add)
            F.update(hm3=hm3, s3b=s3b)

        # ------------------------------------------------ backward of tile t
        # stage A(i): a*(V=C(I+D3W3)) -> v_ps[i]        (PE f32r + C fp8 DR)
        # stage B(i): v2s2 = KB*(V*s2)  fp8             (DVE stt)
        # stage C(i): v_ps[i] = a*U  (+= hi/lo fp8 DR)  (PE)
        # stage D(i): v1s1 = U*s1  bf16                 (DVE stt)
        # stage E(i): jt += J^T  bf16 smalls            (PE)
        SKEW = [(0, 0), (0, 1), (1, 0), (0, 2), (1, 1), (2, 0), (0, 3),
                (1, 2), (2, 1), (3, 0), (1, 3), (2, 2), (3, 1), (4, 0),
                (2, 3), (3, 2), (4, 1), (3, 3), (4, 2), (4, 3)]

        def make_backward(t, F):
            w = t % SUP          # position within supertile
            s = t // SUP         # supertile index
            sq = s % SOLVE_SUPS  # position within solve group
            last = (t == ntiles - 1)
            st = {}

            def start():
                ready = pending[:]
                pending.clear()
                st["ready"] = ready
                if w == 0:
                    cur["jt"] = tps_pool.tile([128, GRPS, 4 * NS], F32,
                                              tag="jtps", name="jt_ps")
                    cur["dt"] = tps_pool.tile([128, GRPS, NS], F32,
                                              tag="dtps", name="dt_ps")
                if sq == 0 and w == 0:
                    grp["msb"] = mpool.tile([128, SROWS, 10], F32, tag="msb", name="msb")
                    grp["vsb"] = mpool.tile([128, SROWS, 4], F32, tag="vsb", name="vsb")
                    grp["csb"] = mpool.tile([128, SROWS, 4], F32, tag="csb", name="csb")
                    grp["jtg"] = jpool.tile([128, SROWS, 4 * NS], BF16, tag="jtg", name="jtg")
                    grp["dtg"] = jpool.tile([128, SROWS, NS], F32, tag="dtg", name="dtg")
                    grp["n"] = 0
                st["jt"], st["dt"] = cur["jt"], cur["dt"]

            def dt_emit():
                # d, transposed: stationary = hm3 chunk, moving = L4
                for g in range(4):
                    grpi = w * 4 + g
                    for k in range(2):
                        nc.tensor.matmul(st["dt"][:, grpi],
                                         F["hm3"][:, k, g * 128:(g + 1) * 128],
                                         L4s[:, k], start=(k == 0), stop=(k == 1))
            st["dt_emit"] = dt_emit

            v_tiles = [None] * 4
            v2s2_t = [None] * 4
            v1s1_t = [None] * 4

            def stage_a(i):
                v_ps = psv_pool.tile([128, 2, TN], F32, tag="psv", name=f"vps{i}")
                v_tiles[i] = v_ps
                for j in range(2):
                    for k in range(2):
                        nc.tensor.matmul(v_ps[:, j],
                                         M3s[:, k, i, j * 128:(j + 1) * 128],
                                         F["s3b"][:, k],
                                         start=(k == 0), stop=False)
                    nc.tensor.matmul(v_ps[:, j],
                                     Cb8s[:, :, i, j * 128:(j + 1) * 128],
                                     ones8[:], start=False, stop=False,
                                     perf_mode=DRM)

            def stage_b(i):
                v2s2 = sb.tile([128, 2, TN], FP8, tag="v2s2", name=f"v2s2_{i}")
                v2s2_t[i] = v2s2
                nc.vector.scalar_tensor_tensor(v2s2[:], v_tiles[i][:], KB / ALPHA,
                                               F["s2b"][:], OP.mult, OP.mult)

            def stage_c(i):
                v_ps = v_tiles[i]
                for j in range(2):
                    nc.tensor.matmul(v_ps[:, j], W2his[:, :, j * 128:(j + 1) * 128],
                                     v2s2_t[i][:], start=False, stop=False,
                                     perf_mode=DRM)
                    nc.tensor.matmul(v_ps[:, j], W2los[:, :, j * 128:(j + 1) * 128],
                                     v2s2_t[i][:], start=False, stop=True,
                                     perf_mode=DRM)

            def stage_d(i):
                v1s1 = sb.tile([128, 2, TN], BF16, tag="v1s1", name=f"v1s1_{i}")
                v1s1_t[i] = v1s1
                nc.vector.scalar_tensor_tensor(v1s1[:], v_tiles[i][:], 1.0 / ALPHA,
                                               F["s1b"][:], OP.mult, OP.mult)

            def stage_e(i):
                for g in range(4):
                    grpi = w * 4 + g
                    for k in range(2):
                        nc.tensor.matmul(st["jt"][:, grpi, i * NS:(i + 1) * NS],
                                         v1s1_t[i][:, k, g * 128:(g + 1) * 128],
                                         W1s[:, k], start=(k == 0), stop=(k == 1))

            stages = [stage_a, stage_b, stage_c, stage_d, stage_e]

            def chunk(lo, hi):
                for si, i in SKEW[lo:hi]:
                    stages[si](i)

            st["start"] = start
            st["chunk"] = chunk
            st["finish"] = lambda: finish_backward(t, w, s, sq, last, st)
            return st

        def finish_backward(t, w, s, sq, last, st):
            jt_ps, dt_ps = st["jt"], st["dt"]
            # -------- supertile done: drain PSUM, defer arithmetic
            if w == SUP - 1:
                rows = slice(sq * GRPS, (sq + 1) * GRPS)
                jtg, dtg = grp["jtg"], grp["dtg"]
                msb, vsb = grp["msb"], grp["vsb"]
                nc.scalar.copy(jtg[:, rows], jt_ps[:])
                nc.scalar.copy(dtg[:, rows], dt_ps[:])
                grp["n"] += 1

                def postproc(rows=rows, jtg=jtg, dtg=dtg, msb=msb, vsb=vsb):
                    jts = jtg[:, rows]
                    dts = dtg[:, rows]
                    # Gram products batched by index shift:
                    #   shift0 (0,0)(1,1)(2,2)(3,3) -> msb cols 0..3
                    #   shift1 (0,1)(1,2)(2,3)      -> cols 4..6
                    #   shift2 (0,2)(1,3)           -> cols 7..8
                    #   shift3 (0,3)                -> col  9
                    col0 = [0, 4, 7, 9]
                    for sh in range(4):
                        na = 4 - sh
                        prod = sb2b.tile([128, GRPS, 4, NS], BF16, tag="prod")
                        in0 = jts[:, :, 0:na * NS].rearrange(
                            "p g (a c) -> p g a c", a=na)
                        in1 = jts[:, :, sh * NS:(sh + na) * NS].rearrange(
                            "p g (a c) -> p g a c", a=na)
                        nc.gpsimd.tensor_tensor(prod[:, :, :na], in0, in1,
                                                OP.mult)
                        nc.vector.tensor_reduce(
                            msb[:, rows, col0[sh]:col0[sh] + na],
                            prod[:, :, :na], AX.X, OP.add)
                    prodv = sb2b.tile([128, GRPS, 4, NS], F32, tag="prodv")
                    in0 = jts[:, :, :].rearrange("p g (a c) -> p g a c", a=4)
                    in1 = dts[:, :, :].rearrange(
                        "p g (a c) -> p g a c", a=1).to_broadcast(
                            (128, GRPS, 4, NS))
                    nc.gpsimd.tensor_tensor(prodv[:], in0, in1, OP.mult)
                    nc.vector.tensor_reduce(vsb[:, rows, 0:4], prodv[:],
                                            AX.X, OP.add)

                pending.append(postproc)
                if sq == SOLVE_SUPS - 1 or last:
                    pending.append(make_solve(grp["n"], s, grp["msb"], grp["vsb"],
                                              grp["csb"], grp["jtg"], grp["dtg"]))

            for c in st["ready"]:
                c()

        # ---------------- batched 4x4 solve + combine at end of solve group
        def make_solve(nsup, s, msb, vsb, csb, jtg, dtg):
            def solve():
                R = nsup * GRPS

                def m(i_):
                    return msb[:, :R, i_:i_ + 1]

                def vv(i_):
                    return vsb[:, :R, i_:i_ + 1]

                tt = {}

                def tmp(name):
                    if name not in tt:
                        tt[name] = mp1.tile([128, SROWS, 1], F32, tag=f"t_{name}", name=f"t_{name}")
                    return tt[name][:, :R]

                V = nc.any

                def mul(o, x, y):
                    V.tensor_tensor(o, x, y, OP.mult)

                def sub(o, x, y):
                    V.tensor_tensor(o, x, y, OP.subtract)

                def add(o, x, y):
                    V.tensor_tensor(o, x, y, OP.add)

                # index map: 0:00 1:11 2:22 3:33 4:01 5:12 6:23 7:02 8:13 9:03
                m00, m11, m22, m33, m01, m12, m23, m02, m13, m03 = (m(i_) for i_ in range(10))
                x1, x2 = tmp("x1"), tmp("x2")
                detA = tmp("detA")
                mul(x1, m00, m11); mul(x2, m01, m01); sub(detA, x1, x2)
                u10, u11 = tmp("u10"), tmp("u11")
                mul(x1, m11, vv(0)); mul(x2, m01, vv(1)); sub(u10, x1, x2)
                mul(x1, m00, vv(1)); mul(x2, m01, vv(0)); sub(u11, x1, x2)
                P00, P01, P10, P11 = tmp("P00"), tmp("P01"), tmp("P10"), tmp("P11")
                mul(x1, m11, m02); mul(x2, m01, m12); sub(P00, x1, x2)
                mul(x1, m11, m03); mul(x2, m01, m13); sub(P01, x1, x2)
                mul(x1, m00, m12); mul(x2, m01, m02); sub(P10, x1, x2)
                mul(x1, m00, m13); mul(x2, m01, m03); sub(P11, x1, x2)
                S00, S01, S11 = tmp("S00"), tmp("S01"), tmp("S11")
                mul(x1, m02, P00); mul(x2, m12, P10); add(x1, x1, x2)
                mul(S00, detA, m22); sub(S00, S00, x1)
                mul(x1, m02, P01); mul(x2, m12, P11); add(x1, x1, x2)
                mul(S01, detA, m23); sub(S01, S01, x1)
                mul(x1, m03, P01); mul(x2, m13, P11); add(x1, x1, x2)
                mul(S11, detA, m33); sub(S11, S11, x1)
                w0, w1 = tmp("w0"), tmp("w1")
                mul(x1, m02, u10); mul(x2, m12, u11); add(x1, x1, x2)
                mul(w0, detA, vv(2)); sub(w0, w0, x1)
                mul(x1, m03, u10); mul(x2, m13, u11); add(x1, x1, x2)
                mul(w1, detA, vv(3)); sub(w1, w1, x1)
                detS = tmp("detS")
                mul(x1, S00, S11); mul(x2, S01, S01); sub(detS, x1, x2)
                cw2, cw3 = tmp("cw2"), tmp("cw3")
                mul(x1, S11, w0); mul(x2, S01, w1); sub(cw2, x1, x2)
                mul(x1, S00, w1); mul(x2, S01, w0); sub(cw3, x1, x2)
                q0, q1 = tmp("q0"), tmp("q1")
                mul(x1, P00, cw2); mul(x2, P01, cw3); add(x1, x1, x2)
                mul(q0, u10, detS); sub(q0, q0, x1)
                mul(x1, P10, cw2); mul(x2, P11, cw3); add(x1, x1, x2)
                mul(q1, u11, detS); sub(q1, q1, x1)
                dAS, rAS, rS = tmp("dAS"), tmp("rAS"), tmp("rS")
                mul(dAS, detA, detS)
                nc.vector.reciprocal(rAS, dAS)
                nc.vector.reciprocal(rS, detS)
                mul(csb[:, :R, 0:1], q0, rAS)
                mul(csb[:, :R, 1:2], q1, rAS)
                mul(csb[:, :R, 2:3], cw2, rS)
                mul(csb[:, :R, 3:4], cw3, rS)

                # combine + write out, whole solve group at once
                s_base = s - (nsup - 1)
                R2 = nsup * GRPS
                acc = sb2b.tile([128, SROWS, NS], F32, tag="acc")
                ctmp = sb2b.tile([128, SROWS, NS], F32, tag="ctmp")
                G = nc.gpsimd
                for a in range(4):
                    cb = csb[:, :R2, a:a + 1].to_broadcast((128, R2, NS))
                    G.tensor_tensor(ctmp[:, :R2], cb,
                                    jtg[:, :R2, a * NS:(a + 1) * NS], OP.mult)
                    if a == 0:
                        G.tensor_tensor(acc[:, :R2], dtg[:, :R2], ctmp[:, :R2],
                                        OP.subtract)
                    else:
                        G.tensor_tensor(acc[:, :R2], acc[:, :R2], ctmp[:, :R2],
                                        OP.subtract)
                eb = s_base * SUP * TN
                nc.sync.dma_start(
                    out_d[eb:eb + R2 * 128, :].rearrange(
                        "(g p) m -> p g m", p=128),
                    acc[:, :R2])

            return solve

        # ------------------------------------------------ main loop
        # forward(t+1) is emitted before backward(t): the in-order PE stream
        # gets next-tile matmul work to chew on while tile t's activation
        # chain completes, and vice versa.
        F = None
        for t in range(ntiles + 1):
            Fn = fwd_part1(t) if t < ntiles else None
            B = make_backward(t - 1, F) if F is not None else None
            if B:
                B["start"]()
                B["chunk"](0, 3)          # A0; A1, B0
            if Fn:
                fwd_part2(t, Fn)
            if B:
                B["chunk"](3, 10)         # A2,B1,C0; A3,B2,C1,D0
                B["dt_emit"]()
            if Fn:
                fwd_part3(t, Fn)
            if B:
                B["chunk"](10, 20)        # B3,C2,D1,E0; C3,D2,E1; D3,E2; E3
                B["finish"]()
            F = Fn
        for c in pending:
            c()

    _split_multi_waits(nc)
    return nc


# ---------------------------------------------------------------- entry point
def kernel(zstates, W1, W2, W3, W4):
    from concourse.bass_utils import run_bass_kernel_spmd

    key = "full"
    if key not in _cache:
        _cache[key] = _build()
    nc = _cache[key]

    wm = _prep_weights(W1, W2, W3, W4)
    z = np.asarray(zstates, np.float32).reshape(NCORES, BP, NS)
    in_maps = [
        {**wm, "zT": np.ascontiguousarray(z[c].T)} for c in range(NCORES)
    ]
    res = run_bass_kernel_spmd(nc, in_maps, core_ids=list(range(NCORES)))
    return np.concatenate([res.results[c]["out"] for c in range(NCORES)], axis=0)



# revision 39
# speedup vs baseline: 1.0718x; 1.0718x over previous
"""nn_CoMet Trainium2 kernel.

Math (per batch element s in R^16):
  MLP: h1 = logsig(s@W1.T); h2 = h1 + logsig(h1@W2.T); h3 = h2 + logsig(h2@W3.T)
       nnout = h3@W4.T ; d = nnout[:16]; com-rows C = W4[16:20]
  J = d com/d s  (reverse mode through the MLP, 4 rows)
  out = d - J^T (J J^T)^{-1} J d      (== the QR-based projection in the reference)

Kernel design (per core, batch 32768, hidden-major layout [hidden on partitions,
elements on free axis], tiles of 512 elements):
  - forward matmuls in float32r (full-rate fp32-ish PE mode), activations via the
    natural_log_exp table set only:  e=Exp(-a), Lg=Ln(e+1) (= -logsig(a)),
    r=Exp(-Lg), s=e*r (= sig(-a)).  hm_k = -h_k accumulates Lg terms; signs are
    folded into the (host-negated) weights.
  - backward: V accumulates IN PSUM: t3 = s3 @ diag(C_i)W3, then U = (V2*s2)@W2
    accumulated into the same bank; the "+C_i" term is applied on the fly by
    scalar_tensor_tensor (per-partition scalar) when reading PSUM.
  - J and d are produced already transposed (element-major) by using the batch
    tile as the *stationary* matmul operand, so the 4x4 normal-equation solve
    runs on [128 elems, groups, comps] tiles with full lane utilization.
"""

import numpy as np
import ml_dtypes

import concourse.bass as bass
import concourse.mybir as mybir
import concourse.tile as tile

NCORES = 8
B = 262144
BP = B // NCORES          # 32768 per core
NS = 16
NH = 256
NCOM = 4
TN = 512                  # elements per pipeline tile
NTILES = BP // TN         # 64
SUP = 2                   # tiles per supertile (transpose/postproc unit) -> 1024 elems
GRPS = SUP * TN // 128    # 8 groups of 128 elems per supertile
SOLVE_SUPS = 8            # supertiles per batched solve -> 8192 elems
SROWS = SOLVE_SUPS * GRPS # 64

F32 = mybir.dt.float32
F32R = mybir.dt.float32r
BF16 = mybir.dt.bfloat16
FP8 = mybir.dt.float8e4
AF = mybir.ActivationFunctionType
OP = mybir.AluOpType
AX = mybir.AxisListType
DRM = mybir.MatmulPerfMode.DoubleRow

_cache = {}


# ---------------------------------------------------------------- drain patch
def _patch_tile_drain():
    """walrus in this container rejects >1 sem wait on the TileContext final
    drain ("Too many sync wait commands"); split the waits across several
    drain instructions (1 wait each)."""
    if getattr(tile.TileContext, "_comet_patched", False):
        return
    from concourse.vector_clock import ScopedClock

    def _drain_and_barrier(self, tick_clock, wait_clock):
        nc = self.nc
        drain_inst = nc.sync.drain()
        wait_clock.add_sem_waits(
            drain_inst.ins, ScopedClock({None: tick_clock.global_clock})
        )
        si = drain_inst.ins.sync_info
        waits = list(si.on_wait) if si is not None and si.on_wait else []
        if len(waits) > 1:
            si.on_wait = waits[:1]
            for w in waits[1:]:
                extra = nc.sync.drain()
                esi = extra.ins.sync_info
                if esi is None:
                    import bass_rust
                    extra.ins.sync_info = bass_rust.SyncInfo(
                        on_wait=[w], on_update=[]
                    )
                else:
                    esi.on_wait = list(esi.on_wait or []) + [w]
        nc.all_engine_barrier()
        assert self.sems is not None
        popped = nc._tile_sem_poison_stack.pop()
        assert popped is self._sem_poison
        nc.clear_and_free_semaphores(list(self.sems.allocated().values()))
        nc.all_engine_barrier()

    tile.TileContext._drain_and_barrier = _drain_and_barrier
    tile.TileContext._comet_patched = True


def _split_multi_waits(nc):
    """This container's walrus rejects instructions with more than one sync
    wait command.  Hoist extra waits onto injected same-engine NOPs placed
    immediately before the offending instruction."""
    import bass_rust

    for f in nc.m.functions:
        for b in f.blocks:
            insts = list(b.instructions)
            out, dirty = [], False
            for inst in insts:
                si = inst.sync_info
                waits = list(si.on_wait) if si is not None and si.on_wait else []
                if len(waits) > 1:
                    dirty = True
                    for k, wx in enumerate(waits[:-1]):
                        nop = mybir.InstNoOp(name=f"{inst.name}-ws{k}")
                        nop.engine = inst.engine
                        nop.sync_info = bass_rust.SyncInfo(
                            on_wait=[wx], on_update=[])
                        out.append(nop)
                    si.on_wait = waits[-1:]
                out.append(inst)
            if dirty:
                b.instructions = out


# ------------------------------------------------------- scales (fp8 ranges)
ALPHA = 64.0              # stage_a products (M3, C rows) pre-scale: v_ps = a*V
KB = 8.0                  # v2s2 = (v_ps * KB/ALPHA) . s2 = KB*(V.s2)  [fp8]
SW2 = ALPHA / KB          # W2 fp8 scale so stage_c restores factor ALPHA


# ---------------------------------------------------------------- host prep
def _prep_weights(W1, W2, W3, W4):
    W1 = np.asarray(W1, np.float32)
    W2 = np.asarray(W2, np.float32)
    W3 = np.asarray(W3, np.float32)
    W4 = np.asarray(W4, np.float32)
    W4d, C = W4[:NS], W4[NS:]
    bf = ml_dtypes.bfloat16
    f8 = mybir.dt.np(mybir.dt.float8e4)

    def khalf(a, m):   # [256, m] -> [128, 2, m]
        return np.ascontiguousarray(a.reshape(2, 128, m).transpose(1, 0, 2))

    def hilo(a):  # fp8 hi + residual lo split: dec(hi)+dec(lo) ~ a to ~2^-8
        hi = a.astype(f8)
        lo = (a - hi.astype(np.float32)).astype(f8)
        return hi, lo

    L1 = np.ascontiguousarray(W1.T)                       # [16,256] lhsT for a1
    L2 = khalf(W2.T, NH)                                  # [128,2,256]
    L3 = khalf(W3.T, NH)
    L4 = khalf(W4d.T, NS)                                 # [128,2,16]
    M3 = np.stack([C[i][:, None] * W3 for i in range(4)]) # [4,256,256]
    M3 = np.ascontiguousarray(
        (M3 * ALPHA).reshape(4, 2, 128, NH).transpose(2, 1, 0, 3)  # [128,2,4,256]
    ).astype(np.float32)
    W2hi, W2lo = hilo(khalf(W2 * SW2, NH))                # [128,2,256] DR lhsT for U
    W1b = khalf(W1, NS).astype(bf)                        # [128,2,16] rhs for Jt
    # C rows as a K=1(x2 hi/lo) DR lhsT: [1, 2, 4i, 256] fp8, ALPHA-scaled
    Chi, Clo = hilo(np.ascontiguousarray(
        C[None, :, :] * ALPHA))                           # [1,4,256] each
    Cb8 = np.ascontiguousarray(np.stack([Chi, Clo], axis=1))  # [1,2,4,256]
    return {
        "L1": L1, "L2": L2, "L3": L3, "L4": L4,
        "M3": M3, "W2hi": W2hi, "W2lo": W2lo, "W1b": W1b, "Cb8": Cb8,
    }


# ---------------------------------------------------------------- build
def _build(ntiles=NTILES, a_bufs=1, v_bufs=2):
    _patch_tile_drain()
    nc = bass.Bass()

    zT = nc.dram_tensor("zT", [NS, BP], F32R, kind="ExternalInput")
    dL1 = nc.dram_tensor("L1", [NS, NH], F32R, kind="ExternalInput")
    dL2 = nc.dram_tensor("L2", [128, 2, NH], F32R, kind="ExternalInput")
    dL3 = nc.dram_tensor("L3", [128, 2, NH], F32R, kind="ExternalInput")
    dL4 = nc.dram_tensor("L4", [128, 2, NS], F32R, kind="ExternalInput")
    dM3 = nc.dram_tensor("M3", [128, 2, 4, NH], F32R, kind="ExternalInput")
    dW2hi = nc.dram_tensor("W2hi", [128, 2, NH], FP8, kind="ExternalInput")
    dW2lo = nc.dram_tensor("W2lo", [128, 2, NH], FP8, kind="ExternalInput")
    dW1b = nc.dram_tensor("W1b", [128, 2, NS], BF16, kind="ExternalInput")
    dCb8 = nc.dram_tensor("Cb8", [1, 2, 4, NH], FP8, kind="ExternalInput")
    out_d = nc.dram_tensor("out", [BP, NS], F32, kind="ExternalOutput")

    from contextlib import ExitStack
    with tile.TileContext(nc) as tc, ExitStack() as ctx:
        wpool = ctx.enter_context(tc.tile_pool(name="w", bufs=1))
        sb = ctx.enter_context(tc.tile_pool(name="sb", bufs=3))
        sb2 = ctx.enter_context(tc.tile_pool(name="sb2", bufs=3))
        sb2b = ctx.enter_context(tc.tile_pool(name="sb2b", bufs=2))
        mp1 = ctx.enter_context(tc.tile_pool(name="mp1", bufs=1))
        jpool = ctx.enter_context(tc.tile_pool(name="jp", bufs=2))
        mpool = ctx.enter_context(tc.tile_pool(name="mp", bufs=2))
        # Two PSUM rings chosen so every WAR wait lands on a dependency the
        # consumer already has:  psv (bufs=2) carries [a1, v0..v3] per tile —
        # a1's bank is freed by sig1 early, and each v-alloc's ring
        # predecessor died a full backward earlier.  psa (bufs=1) alternates
        # [a2, a3]: a2(t) waits sig3(t-1) (long done) and a3(t) waits sig2(t),
        # which gates a3's rhs (lg2) anyway.
        psv_pool = ctx.enter_context(tc.tile_pool(name="psv", bufs=2, space="PSUM"))
        psa_pool = ctx.enter_context(tc.tile_pool(name="psa", bufs=1, space="PSUM"))
        tps_pool = ctx.enter_context(tc.tile_pool(name="tps", bufs=1, space="PSUM"))

        # ---- load constants
        L1s = wpool.tile([NS, NH], F32R)
        nc.sync.dma_start(L1s[:], dL1[:])
        L2s = wpool.tile([128, 2, NH], F32R)
        nc.sync.dma_start(L2s[:], dL2[:])
        L3s = wpool.tile([128, 2, NH], F32R)
        nc.sync.dma_start(L3s[:], dL3[:])
        L4s = wpool.tile([128, 2, NS], F32R)
        nc.sync.dma_start(L4s[:], dL4[:])
        M3s = wpool.tile([128, 2, 4, NH], F32R)
        nc.sync.dma_start(M3s[:], dM3[:])
        W2his = wpool.tile([128, 2, NH], FP8)
        nc.sync.dma_start(W2his[:], dW2hi[:])
        W2los = wpool.tile([128, 2, NH], FP8)
        nc.sync.dma_start(W2los[:], dW2lo[:])
        W1s = wpool.tile([128, 2, NS], BF16)
        nc.sync.dma_start(W1s[:], dW1b[:])
        Cb8s = wpool.tile([1, 2, 4, NH], FP8)
        nc.sync.dma_start(Cb8s[:], dCb8[:])
        ones8 = wpool.tile([1, 2, TN], FP8)
        nc.vector.memset(ones8[:], 1.0)

        cur = {}       # current supertile PSUM accumulators
        grp = {}       # current solve-group SBUF state
        pending = []   # deferred postproc/solve emission closures

        # ------------------------------------------------ forward pieces
        def act_block(a_psum, layer, sdt=F32):
            # s = sigmoid(-a); h-term = logsig(a) = ln(1 - s)
            sg = sb2.tile([128, 2, TN], sdt, tag=f"s{layer}", name=f"s{layer}")
            nc.scalar.activation(sg[:], a_psum[:], AF.Sigmoid, scale=-1.0)
            lg = sb2.tile([128, 2, TN], F32R, tag=f"lg{layer}", name=f"lg{layer}")
            nc.scalar.activation(lg[:], sg[:], AF.Ln, bias=1.0, scale=-1.0)
            return lg, sg

        def fwd_part1(t):
            e0 = t * TN
            zt = sb2.tile([NS, TN], F32R, tag="zt")
            nc.sync.dma_start(zt[:], zT[:, e0:e0 + TN])
            a_ps = psv_pool.tile([128, 2, TN], F32, tag="psv", name="a1ps")
            for j in range(2):
                nc.tensor.matmul(a_ps[:, j], L1s[:, j * 128:(j + 1) * 128],
                                 zt[:], start=True, stop=True)
            hm1, s1b = act_block(a_ps, 1)   # hm1 == h1 = logsig(a1)
            return dict(hm1=hm1, s1b=s1b)

        def fwd_part2(t, F):
            a_ps = psa_pool.tile([128, 2, TN], F32, tag="psa", name="a2ps")
            for j in range(2):
                for k in range(2):
                    nc.tensor.matmul(a_ps[:, j],
                                     L2s[:, k, j * 128:(j + 1) * 128],
                                     F["hm1"][:, k], start=(k == 0), stop=(k == 1))
            lg2, s2b = act_block(a_ps, 2)
            hm2 = sb2.tile([128, 2, TN], F32R, tag="hm2")
            nc.gpsimd.tensor_tensor(hm2[:], F["hm1"][:], lg2[:], OP.add)
            F.update(hm2=hm2, s2b=s2b)

        def fwd_part3(t, F):
            a_ps = psa_pool.tile([128, 2, TN], F32, tag="psa", name="a3ps")
            for j in range(2):
                for k in range(2):
                    nc.tensor.matmul(a_ps[:, j],
                                     L3s[:, k, j * 128:(j + 1) * 128],
                                     F["hm2"][:, k], start=(k == 0), stop=(k == 1))
            lg3, s3b = act_block(a_ps, 3, sdt=F32R)
            hm3 = sb2.tile([128, 2, TN], F32R, tag="hm3")
            nc.gpsimd.tensor_tensor(hm3[:], F["hm2"][:], lg3[:], OP.add)
            F.update(hm3=hm3, s3b=s3b)

        # ------------------------------------------------ backward of tile t
        # stage A(i): a*(V=C(I+D3W3)) -> v_ps[i]        (PE f32r + C fp8 DR)
        # stage B(i): v2s2 = KB*(V*s2)  fp8             (DVE stt)
        # stage C(i): v_ps[i] = a*U  (+= hi/lo fp8 DR)  (PE)
        # stage D(i): v1s1 = U*s1  bf16                 (DVE stt)
        # stage E(i): jt += J^T  bf16 smalls            (PE)
        SKEW = [(0, 0), (1, 0), (0, 1), (1, 1), (2, 0), (0, 2), (1, 2),
                (2, 1), (3, 0), (0, 3), (1, 3), (2, 2), (3, 1), (4, 0),
                (2, 3), (3, 2), (4, 1), (3, 3), (4, 2), (4, 3)]

        def make_backward(t, F):
            w = t % SUP          # position within supertile
            s = t // SUP         # supertile index
            sq = s % SOLVE_SUPS  # position within solve group
            last = (t == ntiles - 1)
            st = {}

            def start():
                ready = pending[:]
                pending.clear()
                st["ready"] = ready
                if w == 0:
                    cur["jt"] = tps_pool.tile([128, GRPS, 4 * NS], F32,
                                              tag="jtps", name="jt_ps")
                    cur["dt"] = tps_pool.tile([128, GRPS, NS], F32,
                                              tag="dtps", name="dt_ps")
                if sq == 0 and w == 0:
                    grp["msb"] = mpool.tile([128, SROWS, 14], F32, tag="msb", name="msb")
                    grp["vsb"] = grp["msb"]
                    grp["csb"] = mpool.tile([128, SROWS, 4], F32, tag="csb", name="csb")
                    grp["jtg"] = jpool.tile([128, SROWS, 4 * NS], BF16, tag="jtg", name="jtg")
                    grp["dtg"] = jpool.tile([128, SROWS, NS], F32, tag="dtg", name="dtg")
                    grp["n"] = 0
                st["jt"], st["dt"] = cur["jt"], cur["dt"]

            def dt_emit():
                # d, transposed: stationary = hm3 chunk, moving = L4
                for g in range(4):
                    grpi = w * 4 + g
                    for k in range(2):
                        nc.tensor.matmul(st["dt"][:, grpi],
                                         F["hm3"][:, k, g * 128:(g + 1) * 128],
                                         L4s[:, k], start=(k == 0), stop=(k == 1))
            st["dt_emit"] = dt_emit

            v_tiles = [None] * 4
            v2s2_t = [None] * 4
            v1s1_t = [None] * 4

            def stage_a(i):
                v_ps = psv_pool.tile([128, 2, TN], F32, tag="psv", name=f"vps{i}")
                v_tiles[i] = v_ps
                for j in range(2):
                    for k in range(2):
                        nc.tensor.matmul(v_ps[:, j],
                                         M3s[:, k, i, j * 128:(j + 1) * 128],
                                         F["s3b"][:, k],
                                         start=(k == 0), stop=False)
                    nc.tensor.matmul(v_ps[:, j],
                                     Cb8s[:, :, i, j * 128:(j + 1) * 128],
                                     ones8[:], start=False, stop=False,
                                     perf_mode=DRM)

            def stage_b(i):
                v2s2 = sb.tile([128, 2, TN], FP8, tag="v2s2", name=f"v2s2_{i}")
                v2s2_t[i] = v2s2
                nc.vector.scalar_tensor_tensor(v2s2[:], v_tiles[i][:], KB / ALPHA,
                                               F["s2b"][:], OP.mult, OP.mult)

            def stage_c(i):
                v_ps = v_tiles[i]
                for j in range(2):
                    nc.tensor.matmul(v_ps[:, j], W2his[:, :, j * 128:(j + 1) * 128],
                                     v2s2_t[i][:], start=False, stop=False,
                                     perf_mode=DRM)
                    nc.tensor.matmul(v_ps[:, j], W2los[:, :, j * 128:(j + 1) * 128],
                                     v2s2_t[i][:], start=False, stop=True,
                                     perf_mode=DRM)

            def stage_d(i):
                v1s1 = sb.tile([128, 2, TN], BF16, tag="v1s1", name=f"v1s1_{i}")
                v1s1_t[i] = v1s1
                nc.vector.scalar_tensor_tensor(v1s1[:], v_tiles[i][:], 1.0 / ALPHA,
                                               F["s1b"][:], OP.mult, OP.mult)

            def stage_e(i):
                for g in range(4):
                    grpi = w * 4 + g
                    for k in range(2):
                        nc.tensor.matmul(st["jt"][:, grpi, i * NS:(i + 1) * NS],
                                         v1s1_t[i][:, k, g * 128:(g + 1) * 128],
                                         W1s[:, k], start=(k == 0), stop=(k == 1))

            stages = [stage_a, stage_b, stage_c, stage_d, stage_e]

            def chunk(lo, hi):
                for si, i in SKEW[lo:hi]:
                    stages[si](i)

            st["start"] = start
            st["chunk"] = chunk
            st["finish"] = lambda: finish_backward(t, w, s, sq, last, st)
            return st

        def finish_backward(t, w, s, sq, last, st):
            jt_ps, dt_ps = st["jt"], st["dt"]
            # -------- supertile done: drain PSUM, defer arithmetic
            if w == SUP - 1:
                rows = slice(sq * GRPS, (sq + 1) * GRPS)
                jtg, dtg = grp["jtg"], grp["dtg"]
                msb, vsb = grp["msb"], grp["vsb"]
                nc.scalar.copy(jtg[:, rows], jt_ps[:])
                nc.scalar.copy(dtg[:, rows], dt_ps[:])
                grp["n"] += 1

                def postproc(rows=rows, jtg=jtg, dtg=dtg, msb=msb, vsb=vsb):
                    jts = jtg[:, rows]
                    dts = dtg[:, rows]
                    # Gram products batched by index shift:
                    #   shift0 (0,0)(1,1)(2,2)(3,3) -> msb cols 0..3
                    #   shift1 (0,1)(1,2)(2,3)      -> cols 4..6
                    #   shift2 (0,2)(1,3)           -> cols 7..8
                    #   shift3 (0,3)                -> col  9
                    col0 = [0, 4, 7, 9]
                    prod = sb2b.tile([128, GRPS, 14, NS], BF16, tag="prod")
                    for sh in range(4):
                        na = 4 - sh
                        in0 = jts[:, :, 0:na * NS].rearrange(
                            "p g (a c) -> p g a c", a=na)
                        in1 = jts[:, :, sh * NS:(sh + na) * NS].rearrange(
                            "p g (a c) -> p g a c", a=na)
                        nc.gpsimd.tensor_tensor(
                            prod[:, :, col0[sh]:col0[sh] + na], in0, in1,
                            OP.mult)
                    in0 = jts[:, :, :].rearrange("p g (a c) -> p g a c", a=4)
                    in1 = dts[:, :, :].rearrange(
                        "p g (a c) -> p g a c", a=1).to_broadcast(
                            (128, GRPS, 4, NS))
                    nc.gpsimd.tensor_tensor(prod[:, :, 10:14], in0, in1, OP.mult)
                    nc.vector.tensor_reduce(msb[:, rows, 0:14], prod[:],
                                            AX.X, OP.add)

                pending.append(postproc)
                if sq == SOLVE_SUPS - 1 or last:
                    pending.append(make_solve(grp["n"], s, grp["msb"], grp["vsb"],
                                              grp["csb"], grp["jtg"], grp["dtg"]))

            for c in st["ready"]:
                c()

        # ---------------- batched 4x4 solve + combine at end of solve group
        def make_solve(nsup, s, msb, vsb, csb, jtg, dtg):
            def solve():
                R = nsup * GRPS

                def m(i_):
                    return msb[:, :R, i_:i_ + 1]

                def vv(i_):
                    return vsb[:, :R, 10 + i_:11 + i_]

                tt = {}

                def tmp(name):
                    if name not in tt:
                        tt[name] = mp1.tile([128, SROWS, 1], F32, tag=f"t_{name}", name=f"t_{name}")
                    return tt[name][:, :R]

                V = nc.any

                def mul(o, x, y):
                    V.tensor_tensor(o, x, y, OP.mult)

                def sub(o, x, y):
                    V.tensor_tensor(o, x, y, OP.subtract)

                def add(o, x, y):
                    V.tensor_tensor(o, x, y, OP.add)

                # index map: 0:00 1:11 2:22 3:33 4:01 5:12 6:23 7:02 8:13 9:03
                m00, m11, m22, m33, m01, m12, m23, m02, m13, m03 = (m(i_) for i_ in range(10))
                x1, x2 = tmp("x1"), tmp("x2")
                detA = tmp("detA")
                mul(x1, m00, m11); mul(x2, m01, m01); sub(detA, x1, x2)
                u10, u11 = tmp("u10"), tmp("u11")
                mul(x1, m11, vv(0)); mul(x2, m01, vv(1)); sub(u10, x1, x2)
                mul(x1, m00, vv(1)); mul(x2, m01, vv(0)); sub(u11, x1, x2)
                P00, P01, P10, P11 = tmp("P00"), tmp("P01"), tmp("P10"), tmp("P11")
                mul(x1, m11, m02); mul(x2, m01, m12); sub(P00, x1, x2)
                mul(x1, m11, m03); mul(x2, m01, m13); sub(P01, x1, x2)
                mul(x1, m00, m12); mul(x2, m01, m02); sub(P10, x1, x2)
                mul(x1, m00, m13); mul(x2, m01, m03); sub(P11, x1, x2)
                S00, S01, S11 = tmp("S00"), tmp("S01"), tmp("S11")
                mul(x1, m02, P00); mul(x2, m12, P10); add(x1, x1, x2)
                mul(S00, detA, m22); sub(S00, S00, x1)
                mul(x1, m02, P01); mul(x2, m12, P11); add(x1, x1, x2)
                mul(S01, detA, m23); sub(S01, S01, x1)
                mul(x1, m03, P01); mul(x2, m13, P11); add(x1, x1, x2)
                mul(S11, detA, m33); sub(S11, S11, x1)
                w0, w1 = tmp("w0"), tmp("w1")
                mul(x1, m02, u10); mul(x2, m12, u11); add(x1, x1, x2)
                mul(w0, detA, vv(2)); sub(w0, w0, x1)
                mul(x1, m03, u10); mul(x2, m13, u11); add(x1, x1, x2)
                mul(w1, detA, vv(3)); sub(w1, w1, x1)
                detS = tmp("detS")
                mul(x1, S00, S11); mul(x2, S01, S01); sub(detS, x1, x2)
                cw2, cw3 = tmp("cw2"), tmp("cw3")
                mul(x1, S11, w0); mul(x2, S01, w1); sub(cw2, x1, x2)
                mul(x1, S00, w1); mul(x2, S01, w0); sub(cw3, x1, x2)
                q0, q1 = tmp("q0"), tmp("q1")
                mul(x1, P00, cw2); mul(x2, P01, cw3); add(x1, x1, x2)
                mul(q0, u10, detS); sub(q0, q0, x1)
                mul(x1, P10, cw2); mul(x2, P11, cw3); add(x1, x1, x2)
                mul(q1, u11, detS); sub(q1, q1, x1)
                dAS, rAS, rS = tmp("dAS"), tmp("rAS"), tmp("rS")
                mul(dAS, detA, detS)
                nc.vector.reciprocal(rAS, dAS)
                nc.vector.reciprocal(rS, detS)
                mul(csb[:, :R, 0:1], q0, rAS)
                mul(csb[:, :R, 1:2], q1, rAS)
                mul(csb[:, :R, 2:3], cw2, rS)
                mul(csb[:, :R, 3:4], cw3, rS)

                # combine + write out, whole solve group at once
                s_base = s - (nsup - 1)
                R2 = nsup * GRPS
                acc = sb2b.tile([128, SROWS, NS], F32, tag="acc")
                ctmp = sb2b.tile([128, SROWS, NS], F32, tag="ctmp")
                G = nc.gpsimd
                for a in range(4):
                    cb = csb[:, :R2, a:a + 1].to_broadcast((128, R2, NS))
                    G.tensor_tensor(ctmp[:, :R2], cb,
                                    jtg[:, :R2, a * NS:(a + 1) * NS], OP.mult)
                    if a == 0:
                        G.tensor_tensor(acc[:, :R2], dtg[:, :R2], ctmp[:, :R2],
                                        OP.subtract)
                    else:
                        G.tensor_tensor(acc[:, :R2], acc[:, :R2], ctmp[:, :R2],
                                        OP.subtract)
                eb = s_base * SUP * TN
                nc.sync.dma_start(
                    out_d[eb:eb + R2 * 128, :].rearrange(
                        "(g p) m -> p g m", p=128),
                    acc[:, :R2])

            return solve

        # ------------------------------------------------ main loop
        # forward(t+1) is emitted before backward(t): the in-order PE stream
        # gets next-tile matmul work to chew on while tile t's activation
        # chain completes, and vice versa.
        F = None
        for t in range(ntiles + 1):
            Fn = fwd_part1(t) if t < ntiles else None
            B = make_backward(t - 1, F) if F is not None else None
            if B:
                B["start"]()
                B["chunk"](0, 4)          # A0,B0,A1,B1
            if Fn:
                fwd_part2(t, Fn)
            if B:
                B["chunk"](4, 13)         # C0,A2,B2,C1,D0,A3,B3,C2,D1
                B["dt_emit"]()
            if Fn:
                fwd_part3(t, Fn)
            if B:
                B["chunk"](13, 20)        # E0,C3,D2,E1,D3,E2,E3
                B["finish"]()
            F = Fn
        for c in pending:
            c()

    _split_multi_waits(nc)
    return nc


# ---------------------------------------------------------------- entry point
def kernel(zstates, W1, W2, W3, W4):
    from concourse.bass_utils import run_bass_kernel_spmd

    key = "full"
    if key not in _cache:
        _cache[key] = _build()
    nc = _cache[key]

    wm = _prep_weights(W1, W2, W3, W4)
    z = np.asarray(zstates, np.float32).reshape(NCORES, BP, NS)
    in_maps = [
        {**wm, "zT": np.ascontiguousarray(z[c].T)} for c in range(NCORES)
    ]
    res = run_bass_kernel_spmd(nc, in_maps, core_ids=list(range(NCORES)))
    return np.concatenate([res.results[c]["out"] for c in range(NCORES)], axis=0)



# revision 48
# speedup vs baseline: 1.0917x; 1.0186x over previous
"""nn_CoMet Trainium2 kernel.

Math (per batch element s in R^16):
  MLP: h1 = logsig(s@W1.T); h2 = h1 + logsig(h1@W2.T); h3 = h2 + logsig(h2@W3.T)
       nnout = h3@W4.T ; d = nnout[:16]; com-rows C = W4[16:20]
  J = d com/d s  (reverse mode through the MLP, 4 rows)
  out = d - J^T (J J^T)^{-1} J d      (== the QR-based projection in the reference)

Kernel design (per core, batch 32768, hidden-major layout [hidden on partitions,
elements on free axis], tiles of 512 elements):
  - forward matmuls in float32r (full-rate fp32-ish PE mode), activations via the
    natural_log_exp table set only:  e=Exp(-a), Lg=Ln(e+1) (= -logsig(a)),
    r=Exp(-Lg), s=e*r (= sig(-a)).  hm_k = -h_k accumulates Lg terms; signs are
    folded into the (host-negated) weights.
  - backward: V accumulates IN PSUM: t3 = s3 @ diag(C_i)W3, then U = (V2*s2)@W2
    accumulated into the same bank; the "+C_i" term is applied on the fly by
    scalar_tensor_tensor (per-partition scalar) when reading PSUM.
  - J and d are produced already transposed (element-major) by using the batch
    tile as the *stationary* matmul operand, so the 4x4 normal-equation solve
    runs on [128 elems, groups, comps] tiles with full lane utilization.
"""

import numpy as np
import ml_dtypes

import concourse.bass as bass
import concourse.mybir as mybir
import concourse.tile as tile

NCORES = 8
B = 262144
BP = B // NCORES          # 32768 per core
NS = 16
NH = 256
NCOM = 4
TN = 512                  # elements per pipeline tile
NTILES = BP // TN         # 64
SUP = 2                   # tiles per supertile (transpose/postproc unit) -> 1024 elems
GRPS = SUP * TN // 128    # 8 groups of 128 elems per supertile
SOLVE_SUPS = 8            # supertiles per batched solve -> 8192 elems
SROWS = SOLVE_SUPS * GRPS # 64

F32 = mybir.dt.float32
F32R = mybir.dt.float32r
BF16 = mybir.dt.bfloat16
FP8 = mybir.dt.float8e4
AF = mybir.ActivationFunctionType
OP = mybir.AluOpType
AX = mybir.AxisListType
DRM = mybir.MatmulPerfMode.DoubleRow

_cache = {}


# ---------------------------------------------------------------- drain patch
def _patch_tile_drain():
    """walrus in this container rejects >1 sem wait on the TileContext final
    drain ("Too many sync wait commands"); split the waits across several
    drain instructions (1 wait each)."""
    if getattr(tile.TileContext, "_comet_patched", False):
        return
    from concourse.vector_clock import ScopedClock

    def _drain_and_barrier(self, tick_clock, wait_clock):
        nc = self.nc
        drain_inst = nc.sync.drain()
        wait_clock.add_sem_waits(
            drain_inst.ins, ScopedClock({None: tick_clock.global_clock})
        )
        si = drain_inst.ins.sync_info
        waits = list(si.on_wait) if si is not None and si.on_wait else []
        if len(waits) > 1:
            si.on_wait = waits[:1]
            for w in waits[1:]:
                extra = nc.sync.drain()
                esi = extra.ins.sync_info
                if esi is None:
                    import bass_rust
                    extra.ins.sync_info = bass_rust.SyncInfo(
                        on_wait=[w], on_update=[]
                    )
                else:
                    esi.on_wait = list(esi.on_wait or []) + [w]
        nc.all_engine_barrier()
        assert self.sems is not None
        popped = nc._tile_sem_poison_stack.pop()
        assert popped is self._sem_poison
        nc.clear_and_free_semaphores(list(self.sems.allocated().values()))
        nc.all_engine_barrier()

    tile.TileContext._drain_and_barrier = _drain_and_barrier
    tile.TileContext._comet_patched = True


def _split_multi_waits(nc):
    """This container's walrus rejects instructions with more than one sync
    wait command.  Hoist extra waits onto injected same-engine NOPs placed
    immediately before the offending instruction."""
    import bass_rust

    for f in nc.m.functions:
        for b in f.blocks:
            insts = list(b.instructions)
            out, dirty = [], False
            for inst in insts:
                si = inst.sync_info
                waits = list(si.on_wait) if si is not None and si.on_wait else []
                if len(waits) > 1:
                    dirty = True
                    for k, wx in enumerate(waits[:-1]):
                        nop = mybir.InstNoOp(name=f"{inst.name}-ws{k}")
                        nop.engine = inst.engine
                        nop.sync_info = bass_rust.SyncInfo(
                            on_wait=[wx], on_update=[])
                        out.append(nop)
                    si.on_wait = waits[-1:]
                out.append(inst)
            if dirty:
                b.instructions = out


# ------------------------------------------------------- scales (fp8 ranges)
ALPHA = 64.0              # stage_a products (M3, C rows) pre-scale: v_ps = a*V
KB = 8.0                  # v2s2 = (v_ps * KB/ALPHA) . s2 = KB*(V.s2)  [fp8]
SW2 = ALPHA / KB          # W2 fp8 scale so stage_c restores factor ALPHA


# ---------------------------------------------------------------- host prep
def _prep_weights(W1, W2, W3, W4):
    W1 = np.asarray(W1, np.float32)
    W2 = np.asarray(W2, np.float32)
    W3 = np.asarray(W3, np.float32)
    W4 = np.asarray(W4, np.float32)
    W4d, C = W4[:NS], W4[NS:]
    bf = ml_dtypes.bfloat16
    f8 = mybir.dt.np(mybir.dt.float8e4)

    def khalf(a, m):   # [256, m] -> [128, 2, m]
        return np.ascontiguousarray(a.reshape(2, 128, m).transpose(1, 0, 2))

    def hilo(a):  # fp8 hi + residual lo split: dec(hi)+dec(lo) ~ a to ~2^-8
        hi = a.astype(f8)
        lo = (a - hi.astype(np.float32)).astype(f8)
        return hi, lo

    L1 = np.ascontiguousarray(W1.T)                       # [16,256] lhsT for a1
    L2 = khalf(W2.T, NH)                                  # [128,2,256]
    L3 = khalf(W3.T, NH)
    L4 = khalf(W4d.T, NS)                                 # [128,2,16]
    M3 = np.stack([C[i][:, None] * W3 for i in range(4)]) # [4,256,256]
    M3 = np.ascontiguousarray(
        (M3 * ALPHA).reshape(4, 2, 128, NH).transpose(2, 1, 0, 3)  # [128,2,4,256]
    ).astype(np.float32)
    W2hi, W2lo = hilo(khalf(W2 * SW2, NH))                # [128,2,256] DR lhsT for U
    W1b = khalf(W1, NS).astype(bf)                        # [128,2,16] rhs for Jt
    # C rows as a K=1(x2 hi/lo) DR lhsT: [1, 2, 4i, 256] fp8, ALPHA-scaled
    Chi, Clo = hilo(np.ascontiguousarray(
        C[None, :, :] * ALPHA))                           # [1,4,256] each
    Cb8 = np.ascontiguousarray(np.stack([Chi, Clo], axis=1))  # [1,2,4,256]
    return {
        "L1": L1, "L2": L2, "L3": L3, "L4": L4,
        "M3": M3, "W2hi": W2hi, "W2lo": W2lo, "W1b": W1b, "Cb8": Cb8,
    }


# ---------------------------------------------------------------- build
def _build(ntiles=NTILES, a_bufs=1, v_bufs=2):
    _patch_tile_drain()
    nc = bass.Bass()

    zT = nc.dram_tensor("zT", [NS, BP], F32R, kind="ExternalInput")
    dL1 = nc.dram_tensor("L1", [NS, NH], F32R, kind="ExternalInput")
    dL2 = nc.dram_tensor("L2", [128, 2, NH], F32R, kind="ExternalInput")
    dL3 = nc.dram_tensor("L3", [128, 2, NH], F32R, kind="ExternalInput")
    dL4 = nc.dram_tensor("L4", [128, 2, NS], F32R, kind="ExternalInput")
    dM3 = nc.dram_tensor("M3", [128, 2, 4, NH], F32R, kind="ExternalInput")
    dW2hi = nc.dram_tensor("W2hi", [128, 2, NH], FP8, kind="ExternalInput")
    dW2lo = nc.dram_tensor("W2lo", [128, 2, NH], FP8, kind="ExternalInput")
    dW1b = nc.dram_tensor("W1b", [128, 2, NS], BF16, kind="ExternalInput")
    dCb8 = nc.dram_tensor("Cb8", [1, 2, 4, NH], FP8, kind="ExternalInput")
    out_d = nc.dram_tensor("out", [BP, NS], F32, kind="ExternalOutput")

    from contextlib import ExitStack
    with tile.TileContext(nc) as tc, ExitStack() as ctx:
        wpool = ctx.enter_context(tc.tile_pool(name="w", bufs=1))
        sb = ctx.enter_context(tc.tile_pool(name="sb", bufs=3))
        sb2 = ctx.enter_context(tc.tile_pool(name="sb2", bufs=4))
        sb2b = ctx.enter_context(tc.tile_pool(name="sb2b", bufs=2))
        mp1 = ctx.enter_context(tc.tile_pool(name="mp1", bufs=1))
        jpool = ctx.enter_context(tc.tile_pool(name="jp", bufs=2))
        mpool = ctx.enter_context(tc.tile_pool(name="mp", bufs=2))
        # Two PSUM rings chosen so every WAR wait lands on a dependency the
        # consumer already has:  psv (bufs=2) carries [a1, v0..v3] per tile —
        # a1's bank is freed by sig1 early, and each v-alloc's ring
        # predecessor died a full backward earlier.  psa (bufs=1) alternates
        # [a2, a3]: a2(t) waits sig3(t-1) (long done) and a3(t) waits sig2(t),
        # which gates a3's rhs (lg2) anyway.
        psv_pool = ctx.enter_context(tc.tile_pool(name="psv", bufs=2, space="PSUM"))
        psa_pool = ctx.enter_context(tc.tile_pool(name="psa", bufs=1, space="PSUM"))
        tps_pool = ctx.enter_context(tc.tile_pool(name="tps", bufs=1, space="PSUM"))

        # ---- load constants
        L1s = wpool.tile([NS, NH], F32R)
        nc.sync.dma_start(L1s[:], dL1[:])
        L2s = wpool.tile([128, 2, NH], F32R)
        nc.sync.dma_start(L2s[:], dL2[:])
        L3s = wpool.tile([128, 2, NH], F32R)
        nc.sync.dma_start(L3s[:], dL3[:])
        L4s = wpool.tile([128, 2, NS], F32R)
        nc.sync.dma_start(L4s[:], dL4[:])
        M3s = wpool.tile([128, 2, 4, NH], F32R)
        nc.sync.dma_start(M3s[:], dM3[:])
        W2his = wpool.tile([128, 2, NH], FP8)
        nc.sync.dma_start(W2his[:], dW2hi[:])
        W2los = wpool.tile([128, 2, NH], FP8)
        nc.sync.dma_start(W2los[:], dW2lo[:])
        W1s = wpool.tile([128, 2, NS], BF16)
        nc.sync.dma_start(W1s[:], dW1b[:])
        Cb8s = wpool.tile([1, 2, 4, NH], FP8)
        nc.sync.dma_start(Cb8s[:], dCb8[:])
        ones8 = wpool.tile([1, 2, TN], FP8)
        nc.vector.memset(ones8[:], 1.0)

        cur = {}       # current supertile PSUM accumulators
        grp = {}       # current solve-group SBUF state
        pending = []   # deferred postproc/solve emission closures

        # ------------------------------------------------ forward pieces
        def act_block(a_psum, layer, sdt=F32):
            # s = sigmoid(-a); h-term = logsig(a) = ln(1 - s)
            sg = sb2.tile([128, 2, TN], sdt, tag=f"s{layer}", name=f"s{layer}")
            nc.scalar.activation(sg[:], a_psum[:], AF.Sigmoid, scale=-1.0)
            lg = sb2.tile([128, 2, TN], F32R, tag=f"lg{layer}", name=f"lg{layer}")
            nc.scalar.activation(lg[:], sg[:], AF.Ln, bias=1.0, scale=-1.0)
            return lg, sg

        def fwd_part1(t):
            e0 = t * TN
            zt = sb2.tile([NS, TN], F32R, tag="zt")
            nc.sync.dma_start(zt[:], zT[:, e0:e0 + TN])
            a_ps = psv_pool.tile([128, 2, TN], F32, tag="psv", name="a1ps")
            for j in range(2):
                nc.tensor.matmul(a_ps[:, j], L1s[:, j * 128:(j + 1) * 128],
                                 zt[:], start=True, stop=True)
            hm1, s1b = act_block(a_ps, 1)   # hm1 == h1 = logsig(a1)
            return dict(hm1=hm1, s1b=s1b)

        def fwd_part2(t, F):
            a_ps = psa_pool.tile([128, 2, TN], F32, tag="psa", name="a2ps")
            for j in range(2):
                for k in range(2):
                    nc.tensor.matmul(a_ps[:, j],
                                     L2s[:, k, j * 128:(j + 1) * 128],
                                     F["hm1"][:, k], start=(k == 0), stop=(k == 1))
            lg2, s2b = act_block(a_ps, 2)
            hm2 = sb2.tile([128, 2, TN], F32R, tag="hm2")
            nc.gpsimd.tensor_tensor(hm2[:], F["hm1"][:], lg2[:], OP.add)
            F.update(hm2=hm2, s2b=s2b)

        def fwd_part3(t, F):
            a_ps = psa_pool.tile([128, 2, TN], F32, tag="psa", name="a3ps")
            for j in range(2):
                for k in range(2):
                    nc.tensor.matmul(a_ps[:, j],
                                     L3s[:, k, j * 128:(j + 1) * 128],
                                     F["hm2"][:, k], start=(k == 0), stop=(k == 1))
            lg3, s3b = act_block(a_ps, 3, sdt=F32R)
            hm3 = sb2.tile([128, 2, TN], F32R, tag="hm3")
            nc.gpsimd.tensor_tensor(hm3[:], F["hm2"][:], lg3[:], OP.add)
            F.update(hm3=hm3, s3b=s3b)

        # ------------------------------------------------ backward of tile t
        # stage A(i): a*(V=C(I+D3W3)) -> v_ps[i]        (PE f32r + C fp8 DR)
        # stage B(i): v2s2 = KB*(V*s2)  fp8             (DVE stt)
        # stage C(i): v_ps[i] = a*U  (+= hi/lo fp8 DR)  (PE)
        # stage D(i): v1s1 = U*s1  bf16                 (DVE stt)
        # stage E(i): jt += J^T  bf16 smalls            (PE)
        SKEW = [(0, 0), (1, 0), (0, 1), (1, 1), (2, 0), (0, 2), (1, 2),
                (2, 1), (3, 0), (0, 3), (1, 3), (2, 2), (3, 1), (4, 0),
                (2, 3), (3, 2), (4, 1), (3, 3), (4, 2), (4, 3)]

        def make_backward(t, F):
            w = t % SUP          # position within supertile
            s = t // SUP         # supertile index
            sq = s % SOLVE_SUPS  # position within solve group
            last = (t == ntiles - 1)
            st = {}

            def start():
                ready = pending[:]
                pending.clear()
                st["ready"] = ready
                if w == 0:
                    cur["jt"] = tps_pool.tile([128, GRPS, 4 * NS], F32,
                                              tag="jtps", name="jt_ps")
                    cur["dt"] = tps_pool.tile([128, GRPS, NS], F32,
                                              tag="dtps", name="dt_ps")
                if sq == 0 and w == 0:
                    grp["msb"] = mpool.tile([128, SROWS, 14], F32, tag="msb", name="msb")
                    grp["vsb"] = grp["msb"]
                    grp["csb"] = mpool.tile([128, SROWS, 4], F32, tag="csb", name="csb")
                    grp["jtg"] = jpool.tile([128, SROWS, 4 * NS], BF16, tag="jtg", name="jtg")
                    grp["dtg"] = jpool.tile([128, SROWS, NS], F32, tag="dtg", name="dtg")
                    grp["n"] = 0
                st["jt"], st["dt"] = cur["jt"], cur["dt"]

            def dt_emit():
                # d, transposed: stationary = hm3 chunk, moving = L4
                for g in range(4):
                    grpi = w * 4 + g
                    for k in range(2):
                        nc.tensor.matmul(st["dt"][:, grpi],
                                         F["hm3"][:, k, g * 128:(g + 1) * 128],
                                         L4s[:, k], start=(k == 0), stop=(k == 1))
            st["dt_emit"] = dt_emit

            v_tiles = [None] * 4
            v2s2_t = [None] * 4
            v1s1_t = [None] * 4

            def stage_a(i):
                v_ps = psv_pool.tile([128, 2, TN], F32, tag="psv", name=f"vps{i}")
                v_tiles[i] = v_ps
                for j in range(2):
                    for k in range(2):
                        nc.tensor.matmul(v_ps[:, j],
                                         M3s[:, k, i, j * 128:(j + 1) * 128],
                                         F["s3b"][:, k],
                                         start=(k == 0), stop=False)
                    nc.tensor.matmul(v_ps[:, j],
                                     Cb8s[:, :, i, j * 128:(j + 1) * 128],
                                     ones8[:], start=False, stop=False,
                                     perf_mode=DRM)

            def stage_b(i):
                v2s2 = sb.tile([128, 2, TN], FP8, tag="v2s2", name=f"v2s2_{i}")
                v2s2_t[i] = v2s2
                nc.vector.scalar_tensor_tensor(v2s2[:], v_tiles[i][:], KB / ALPHA,
                                               F["s2b"][:], OP.mult, OP.mult)

            def stage_c(i):
                v_ps = v_tiles[i]
                for j in range(2):
                    nc.tensor.matmul(v_ps[:, j], W2his[:, :, j * 128:(j + 1) * 128],
                                     v2s2_t[i][:], start=False, stop=False,
                                     perf_mode=DRM)
                    nc.tensor.matmul(v_ps[:, j], W2los[:, :, j * 128:(j + 1) * 128],
                                     v2s2_t[i][:], start=False, stop=True,
                                     perf_mode=DRM)

            def stage_d(i):
                v1s1 = sb.tile([128, 2, TN], BF16, tag="v1s1", name=f"v1s1_{i}")
                v1s1_t[i] = v1s1
                nc.vector.scalar_tensor_tensor(v1s1[:], v_tiles[i][:], 1.0 / ALPHA,
                                               F["s1b"][:], OP.mult, OP.mult)

            def stage_e(i):
                for g in range(4):
                    grpi = w * 4 + g
                    for k in range(2):
                        nc.tensor.matmul(st["jt"][:, grpi, i * NS:(i + 1) * NS],
                                         v1s1_t[i][:, k, g * 128:(g + 1) * 128],
                                         W1s[:, k], start=(k == 0), stop=(k == 1))

            stages = [stage_a, stage_b, stage_c, stage_d, stage_e]

            def chunk(lo, hi):
                for si, i in SKEW[lo:hi]:
                    stages[si](i)

            st["start"] = start
            st["chunk"] = chunk
            st["finish"] = lambda: finish_backward(t, w, s, sq, last, st)
            return st

        def finish_backward(t, w, s, sq, last, st):
            jt_ps, dt_ps = st["jt"], st["dt"]
            # -------- supertile done: drain PSUM, defer arithmetic
            if w == SUP - 1:
                rows = slice(sq * GRPS, (sq + 1) * GRPS)
                jtg, dtg = grp["jtg"], grp["dtg"]
                msb, vsb = grp["msb"], grp["vsb"]
                nc.scalar.copy(jtg[:, rows], jt_ps[:])
                nc.scalar.copy(dtg[:, rows], dt_ps[:])
                grp["n"] += 1

                def postproc(rows=rows, jtg=jtg, dtg=dtg, msb=msb, vsb=vsb):
                    jts = jtg[:, rows]
                    dts = dtg[:, rows]
                    # Gram products batched by index shift:
                    #   shift0 (0,0)(1,1)(2,2)(3,3) -> msb cols 0..3
                    #   shift1 (0,1)(1,2)(2,3)      -> cols 4..6
                    #   shift2 (0,2)(1,3)           -> cols 7..8
                    #   shift3 (0,3)                -> col  9
                    col0 = [0, 4, 7, 9]
                    prod = sb2b.tile([128, GRPS, 14, NS], BF16, tag="prod")
                    for sh in range(4):
                        na = 4 - sh
                        in0 = jts[:, :, 0:na * NS].rearrange(
                            "p g (a c) -> p g a c", a=na)
                        in1 = jts[:, :, sh * NS:(sh + na) * NS].rearrange(
                            "p g (a c) -> p g a c", a=na)
                        nc.gpsimd.tensor_tensor(
                            prod[:, :, col0[sh]:col0[sh] + na], in0, in1,
                            OP.mult)
                    in0 = jts[:, :, :].rearrange("p g (a c) -> p g a c", a=4)
                    in1 = dts[:, :, :].rearrange(
                        "p g (a c) -> p g a c", a=1).to_broadcast(
                            (128, GRPS, 4, NS))
                    nc.gpsimd.tensor_tensor(prod[:, :, 10:14], in0, in1, OP.mult)
                    nc.vector.tensor_reduce(msb[:, rows, 0:14], prod[:],
                                            AX.X, OP.add)

                pending.append(postproc)
                if sq == SOLVE_SUPS - 1 or last:
                    pending.extend(make_solve(grp["n"], s, grp["msb"], grp["vsb"],
                                              grp["csb"], grp["jtg"], grp["dtg"]))

            if st["ready"]:
                st["ready"].pop(0)()
                pending[0:0] = st["ready"]

        # ---------------- batched 4x4 solve + combine at end of solve group
        def make_solve(nsup, s, msb, vsb, csb, jtg, dtg):
            parts = []

            def part(f):
                parts.append(f)
                return f

            R = nsup * GRPS

            def m(i_):
                return msb[:, :R, i_:i_ + 1]

            def vv(i_):
                return vsb[:, :R, 10 + i_:11 + i_]

            tt = {}

            def tmp(name):
                if name not in tt:
                    tt[name] = mp1.tile([128, SROWS, 1], F32, tag=f"t_{name}", name=f"t_{name}")
                return tt[name][:, :R]

            V = nc.any

            def mul(o, x, y):
                V.tensor_tensor(o, x, y, OP.mult)

            def sub(o, x, y):
                V.tensor_tensor(o, x, y, OP.subtract)

            def add(o, x, y):
                V.tensor_tensor(o, x, y, OP.add)

            # index map: 0:00 1:11 2:22 3:33 4:01 5:12 6:23 7:02 8:13 9:03
            def p1():
                m00, m11, m01 = m(0), m(1), m(4)
                x1, x2 = tmp("x1"), tmp("x2")
                detA = tmp("detA")
                mul(x1, m00, m11); mul(x2, m01, m01); sub(detA, x1, x2)
                u10, u11 = tmp("u10"), tmp("u11")
                mul(x1, m11, vv(0)); mul(x2, m01, vv(1)); sub(u10, x1, x2)
                mul(x1, m00, vv(1)); mul(x2, m01, vv(0)); sub(u11, x1, x2)

            def p2():
                m00, m11, m01, m12, m02, m13, m03 = (
                    m(0), m(1), m(4), m(5), m(7), m(8), m(9))
                x1, x2 = tmp("x1"), tmp("x2")
                P00, P01, P10, P11 = tmp("P00"), tmp("P01"), tmp("P10"), tmp("P11")
                mul(x1, m11, m02); mul(x2, m01, m12); sub(P00, x1, x2)
                mul(x1, m11, m03); mul(x2, m01, m13); sub(P01, x1, x2)
                mul(x1, m00, m12); mul(x2, m01, m02); sub(P10, x1, x2)
                mul(x1, m00, m13); mul(x2, m01, m03); sub(P11, x1, x2)

            def p3():
                m22, m33, m12, m23, m02, m13, m03 = (
                    m(2), m(3), m(5), m(6), m(7), m(8), m(9))
                x1, x2 = tmp("x1"), tmp("x2")
                detA = tmp("detA")
                P00, P01, P10, P11 = tmp("P00"), tmp("P01"), tmp("P10"), tmp("P11")
                S00, S01, S11 = tmp("S00"), tmp("S01"), tmp("S11")
                mul(x1, m02, P00); mul(x2, m12, P10); add(x1, x1, x2)
                mul(S00, detA, m22); sub(S00, S00, x1)
                mul(x1, m02, P01); mul(x2, m12, P11); add(x1, x1, x2)
                mul(S01, detA, m23); sub(S01, S01, x1)
                mul(x1, m03, P01); mul(x2, m13, P11); add(x1, x1, x2)
                mul(S11, detA, m33); sub(S11, S11, x1)

            def p4():
                m12, m02, m13, m03 = m(5), m(7), m(8), m(9)
                x1, x2 = tmp("x1"), tmp("x2")
                detA, u10, u11 = tmp("detA"), tmp("u10"), tmp("u11")
                S00, S01, S11 = tmp("S00"), tmp("S01"), tmp("S11")
                w0, w1, detS = tmp("w0"), tmp("w1"), tmp("detS")
                mul(x1, m02, u10); mul(x2, m12, u11); add(x1, x1, x2)
                mul(w0, detA, vv(2)); sub(w0, w0, x1)
                mul(x1, m03, u10); mul(x2, m13, u11); add(x1, x1, x2)
                mul(w1, detA, vv(3)); sub(w1, w1, x1)
                mul(x1, S00, S11); mul(x2, S01, S01); sub(detS, x1, x2)

            def p5():
                x1, x2 = tmp("x1"), tmp("x2")
                u10, u11, detA = tmp("u10"), tmp("u11"), tmp("detA")
                S00, S01, S11 = tmp("S00"), tmp("S01"), tmp("S11")
                P00, P01, P10, P11 = tmp("P00"), tmp("P01"), tmp("P10"), tmp("P11")
                w0, w1, detS = tmp("w0"), tmp("w1"), tmp("detS")
                cw2, cw3 = tmp("cw2"), tmp("cw3")
                mul(x1, S11, w0); mul(x2, S01, w1); sub(cw2, x1, x2)
                mul(x1, S00, w1); mul(x2, S01, w0); sub(cw3, x1, x2)
                q0, q1 = tmp("q0"), tmp("q1")
                mul(x1, P00, cw2); mul(x2, P01, cw3); add(x1, x1, x2)
                mul(q0, u10, detS); sub(q0, q0, x1)
                mul(x1, P10, cw2); mul(x2, P11, cw3); add(x1, x1, x2)
                mul(q1, u11, detS); sub(q1, q1, x1)

            def p6():
                detA, detS = tmp("detA"), tmp("detS")
                q0, q1, cw2, cw3 = tmp("q0"), tmp("q1"), tmp("cw2"), tmp("cw3")
                dAS, rAS, rS = tmp("dAS"), tmp("rAS"), tmp("rS")
                mul(dAS, detA, detS)
                nc.vector.reciprocal(rAS, dAS)
                nc.vector.reciprocal(rS, detS)
                mul(csb[:, :R, 0:1], q0, rAS)
                mul(csb[:, :R, 1:2], q1, rAS)
                mul(csb[:, :R, 2:3], cw2, rS)
                mul(csb[:, :R, 3:4], cw3, rS)

            # combine + write out
            s_base = s - (nsup - 1)
            R2 = nsup * GRPS
            cst = {}

            def comb(a0, a1):
                def f():
                    G = nc.gpsimd
                    if "acc" not in cst:
                        cst["acc"] = sb2b.tile([128, SROWS, NS], F32, tag="acc",
                                               name="acc")
                        cst["ctmp"] = sb2b.tile([128, SROWS, NS], F32,
                                                tag="ctmp", name="ctmp")
                    acc, ctmp = cst["acc"], cst["ctmp"]
                    for a in range(a0, a1):
                        cb = csb[:, :R2, a:a + 1].to_broadcast((128, R2, NS))
                        G.tensor_tensor(ctmp[:, :R2], cb,
                                        jtg[:, :R2, a * NS:(a + 1) * NS],
                                        OP.mult)
                        if a == 0:
                            G.tensor_tensor(acc[:, :R2], dtg[:, :R2],
                                            ctmp[:, :R2], OP.subtract)
                        else:
                            G.tensor_tensor(acc[:, :R2], acc[:, :R2],
                                            ctmp[:, :R2], OP.subtract)
                return f

            def dma_out():
                eb = s_base * SUP * TN
                nc.sync.dma_start(
                    out_d[eb:eb + R2 * 128, :].rearrange(
                        "(g p) m -> p g m", p=128),
                    cst["acc"][:, :R2])

            return [p1, p2, p3, p4, p5, p6, comb(0, 2), comb(2, 4), dma_out]

        # ------------------------------------------------ main loop
        # forward(t+1) is emitted before backward(t): the in-order PE stream
        # gets next-tile matmul work to chew on while tile t's activation
        # chain completes, and vice versa.
        F = None
        for t in range(ntiles + 1):
            Fn = fwd_part1(t) if t < ntiles else None
            B = make_backward(t - 1, F) if F is not None else None
            if B:
                B["start"]()
                B["chunk"](0, 4)          # A0,B0,A1,B1
            if Fn:
                fwd_part2(t, Fn)
            if B:
                B["chunk"](4, 13)         # C0,A2,B2,C1,D0,A3,B3,C2,D1
                B["dt_emit"]()
            if Fn:
                fwd_part3(t, Fn)
            if B:
                B["chunk"](13, 20)        # E0,C3,D2,E1,D3,E2,E3
                B["finish"]()
            F = Fn
        for c in pending:
            c()

    _split_multi_waits(nc)
    return nc


# ---------------------------------------------------------------- entry point
def kernel(zstates, W1, W2, W3, W4):
    from concourse.bass_utils import run_bass_kernel_spmd

    key = "full"
    if key not in _cache:
        _cache[key] = _build()
    nc = _cache[key]

    wm = _prep_weights(W1, W2, W3, W4)
    z = np.asarray(zstates, np.float32).reshape(NCORES, BP, NS)
    in_maps = [
        {**wm, "zT": np.ascontiguousarray(z[c].T)} for c in range(NCORES)
    ]
    res = run_bass_kernel_spmd(nc, in_maps, core_ids=list(range(NCORES)))
    return np.concatenate([res.results[c]["out"] for c in range(NCORES)], axis=0)



# revision 54
# speedup vs baseline: 1.1000x; 1.0076x over previous
"""nn_CoMet Trainium2 kernel.

Math (per batch element s in R^16):
  MLP: h1 = logsig(s@W1.T); h2 = h1 + logsig(h1@W2.T); h3 = h2 + logsig(h2@W3.T)
       nnout = h3@W4.T ; d = nnout[:16]; com-rows C = W4[16:20]
  J = d com/d s  (reverse mode through the MLP, 4 rows)
  out = d - J^T (J J^T)^{-1} J d      (== the QR-based projection in the reference)

Kernel design (per core, batch 32768, hidden-major layout [hidden on partitions,
elements on free axis], tiles of 512 elements):
  - forward matmuls in float32r (full-rate fp32-ish PE mode), activations via the
    natural_log_exp table set only:  e=Exp(-a), Lg=Ln(e+1) (= -logsig(a)),
    r=Exp(-Lg), s=e*r (= sig(-a)).  hm_k = -h_k accumulates Lg terms; signs are
    folded into the (host-negated) weights.
  - backward: V accumulates IN PSUM: t3 = s3 @ diag(C_i)W3, then U = (V2*s2)@W2
    accumulated into the same bank; the "+C_i" term is applied on the fly by
    scalar_tensor_tensor (per-partition scalar) when reading PSUM.
  - J and d are produced already transposed (element-major) by using the batch
    tile as the *stationary* matmul operand, so the 4x4 normal-equation solve
    runs on [128 elems, groups, comps] tiles with full lane utilization.
"""

import numpy as np
import ml_dtypes

import concourse.bass as bass
import concourse.mybir as mybir
import concourse.tile as tile

NCORES = 8
B = 262144
BP = B // NCORES          # 32768 per core
NS = 16
NH = 256
NCOM = 4
TN = 512                  # elements per pipeline tile
NTILES = BP // TN         # 64
SUP = 2                   # tiles per supertile (transpose/postproc unit) -> 1024 elems
GRPS = SUP * TN // 128    # 8 groups of 128 elems per supertile
SOLVE_SUPS = 8            # supertiles per batched solve -> 8192 elems
SROWS = SOLVE_SUPS * GRPS # 64

F32 = mybir.dt.float32
F32R = mybir.dt.float32r
BF16 = mybir.dt.bfloat16
FP8 = mybir.dt.float8e4
AF = mybir.ActivationFunctionType
OP = mybir.AluOpType
AX = mybir.AxisListType
DRM = mybir.MatmulPerfMode.DoubleRow

_cache = {}


# ---------------------------------------------------------------- drain patch
def _patch_tile_drain():
    """walrus in this container rejects >1 sem wait on the TileContext final
    drain ("Too many sync wait commands"); split the waits across several
    drain instructions (1 wait each)."""
    if getattr(tile.TileContext, "_comet_patched", False):
        return
    from concourse.vector_clock import ScopedClock

    def _drain_and_barrier(self, tick_clock, wait_clock):
        nc = self.nc
        drain_inst = nc.sync.drain()
        wait_clock.add_sem_waits(
            drain_inst.ins, ScopedClock({None: tick_clock.global_clock})
        )
        si = drain_inst.ins.sync_info
        waits = list(si.on_wait) if si is not None and si.on_wait else []
        if len(waits) > 1:
            si.on_wait = waits[:1]
            for w in waits[1:]:
                extra = nc.sync.drain()
                esi = extra.ins.sync_info
                if esi is None:
                    import bass_rust
                    extra.ins.sync_info = bass_rust.SyncInfo(
                        on_wait=[w], on_update=[]
                    )
                else:
                    esi.on_wait = list(esi.on_wait or []) + [w]
        nc.all_engine_barrier()
        assert self.sems is not None
        popped = nc._tile_sem_poison_stack.pop()
        assert popped is self._sem_poison
        nc.clear_and_free_semaphores(list(self.sems.allocated().values()))
        nc.all_engine_barrier()

    tile.TileContext._drain_and_barrier = _drain_and_barrier
    tile.TileContext._comet_patched = True


def _split_multi_waits(nc):
    """This container's walrus rejects instructions with more than one sync
    wait command.  Hoist extra waits onto injected same-engine NOPs placed
    immediately before the offending instruction."""
    import bass_rust

    for f in nc.m.functions:
        for b in f.blocks:
            insts = list(b.instructions)
            out, dirty = [], False
            for inst in insts:
                si = inst.sync_info
                waits = list(si.on_wait) if si is not None and si.on_wait else []
                if len(waits) > 1:
                    dirty = True
                    for k, wx in enumerate(waits[:-1]):
                        nop = mybir.InstNoOp(name=f"{inst.name}-ws{k}")
                        nop.engine = inst.engine
                        nop.sync_info = bass_rust.SyncInfo(
                            on_wait=[wx], on_update=[])
                        out.append(nop)
                    si.on_wait = waits[-1:]
                out.append(inst)
            if dirty:
                b.instructions = out


# ------------------------------------------------------- scales (fp8 ranges)
ALPHA = 64.0              # stage_a products (M3, C rows) pre-scale: v_ps = a*V
KB = 8.0                  # v2s2 = (v_ps * KB/ALPHA) . s2 = KB*(V.s2)  [fp8]
SW2 = ALPHA / KB          # W2 fp8 scale so stage_c restores factor ALPHA


# ---------------------------------------------------------------- host prep
def _prep_weights(W1, W2, W3, W4):
    W1 = np.asarray(W1, np.float32)
    W2 = np.asarray(W2, np.float32)
    W3 = np.asarray(W3, np.float32)
    W4 = np.asarray(W4, np.float32)
    W4d, C = W4[:NS], W4[NS:]
    bf = ml_dtypes.bfloat16
    f8 = mybir.dt.np(mybir.dt.float8e4)

    def khalf(a, m):   # [256, m] -> [128, 2, m]
        return np.ascontiguousarray(a.reshape(2, 128, m).transpose(1, 0, 2))

    def hilo(a):  # fp8 hi + residual lo split: dec(hi)+dec(lo) ~ a to ~2^-8
        hi = a.astype(f8)
        lo = (a - hi.astype(np.float32)).astype(f8)
        return hi, lo

    L1 = np.ascontiguousarray(W1.T)                       # [16,256] lhsT for a1
    L2 = khalf(W2.T, NH)                                  # [128,2,256]
    L3 = khalf(W3.T, NH)
    L4 = khalf(W4d.T, NS)                                 # [128,2,16]
    M3 = np.stack([C[i][:, None] * W3 for i in range(4)]) # [4,256,256]
    M3 = np.ascontiguousarray(
        (M3 * ALPHA).reshape(4, 2, 128, NH).transpose(2, 1, 0, 3)  # [128,2,4,256]
    ).astype(np.float32)
    W2hi, W2lo = hilo(khalf(W2 * SW2, NH))                # [128,2,256] DR lhsT for U
    W1b = khalf(W1, NS).astype(bf)                        # [128,2,16] rhs for Jt
    # C rows as a K=1(x2 hi/lo) DR lhsT: [1, 2, 4i, 256] fp8, ALPHA-scaled
    Chi, Clo = hilo(np.ascontiguousarray(
        C[None, :, :] * ALPHA))                           # [1,4,256] each
    Cb8 = np.ascontiguousarray(np.stack([Chi, Clo], axis=1))  # [1,2,4,256]
    return {
        "L1": L1, "L2": L2, "L3": L3, "L4": L4,
        "M3": M3, "W2hi": W2hi, "W2lo": W2lo, "W1b": W1b, "Cb8": Cb8,
    }


# ---------------------------------------------------------------- build
def _build(ntiles=NTILES, a_bufs=1, v_bufs=2):
    _patch_tile_drain()
    nc = bass.Bass()

    zT = nc.dram_tensor("zT", [NS, BP], F32R, kind="ExternalInput")
    dL1 = nc.dram_tensor("L1", [NS, NH], F32R, kind="ExternalInput")
    dL2 = nc.dram_tensor("L2", [128, 2, NH], F32R, kind="ExternalInput")
    dL3 = nc.dram_tensor("L3", [128, 2, NH], F32R, kind="ExternalInput")
    dL4 = nc.dram_tensor("L4", [128, 2, NS], F32R, kind="ExternalInput")
    dM3 = nc.dram_tensor("M3", [128, 2, 4, NH], F32R, kind="ExternalInput")
    dW2hi = nc.dram_tensor("W2hi", [128, 2, NH], FP8, kind="ExternalInput")
    dW2lo = nc.dram_tensor("W2lo", [128, 2, NH], FP8, kind="ExternalInput")
    dW1b = nc.dram_tensor("W1b", [128, 2, NS], BF16, kind="ExternalInput")
    dCb8 = nc.dram_tensor("Cb8", [1, 2, 4, NH], FP8, kind="ExternalInput")
    out_d = nc.dram_tensor("out", [BP, NS], F32, kind="ExternalOutput")

    from contextlib import ExitStack
    with tile.TileContext(nc) as tc, ExitStack() as ctx:
        wpool = ctx.enter_context(tc.tile_pool(name="w", bufs=1))
        sb = ctx.enter_context(tc.tile_pool(name="sb", bufs=3))
        sb2 = ctx.enter_context(tc.tile_pool(name="sb2", bufs=3))
        sb2b = ctx.enter_context(tc.tile_pool(name="sb2b", bufs=2))
        mp1 = ctx.enter_context(tc.tile_pool(name="mp1", bufs=1))
        jpool = ctx.enter_context(tc.tile_pool(name="jp", bufs=2))
        mpool = ctx.enter_context(tc.tile_pool(name="mp", bufs=2))
        # Two PSUM rings chosen so every WAR wait lands on a dependency the
        # consumer already has:  psv (bufs=2) carries [a1, v0..v3] per tile —
        # a1's bank is freed by sig1 early, and each v-alloc's ring
        # predecessor died a full backward earlier.  psa (bufs=1) alternates
        # [a2, a3]: a2(t) waits sig3(t-1) (long done) and a3(t) waits sig2(t),
        # which gates a3's rhs (lg2) anyway.
        psv_pool = ctx.enter_context(tc.tile_pool(name="psv", bufs=2, space="PSUM"))
        psa_pool = ctx.enter_context(tc.tile_pool(name="psa", bufs=1, space="PSUM"))
        tps_pool = ctx.enter_context(tc.tile_pool(name="tps", bufs=1, space="PSUM"))

        # ---- load constants
        L1s = wpool.tile([NS, NH], F32R)
        nc.sync.dma_start(L1s[:], dL1[:])
        L2s = wpool.tile([128, 2, NH], F32R)
        nc.sync.dma_start(L2s[:], dL2[:])
        L3s = wpool.tile([128, 2, NH], F32R)
        nc.sync.dma_start(L3s[:], dL3[:])
        L4s = wpool.tile([128, 2, NS], F32R)
        nc.sync.dma_start(L4s[:], dL4[:])
        M3s = wpool.tile([128, 2, 4, NH], F32R)
        nc.sync.dma_start(M3s[:], dM3[:])
        W2his = wpool.tile([128, 2, NH], FP8)
        nc.sync.dma_start(W2his[:], dW2hi[:])
        W2los = wpool.tile([128, 2, NH], FP8)
        nc.sync.dma_start(W2los[:], dW2lo[:])
        W1s = wpool.tile([128, 2, NS], BF16)
        nc.sync.dma_start(W1s[:], dW1b[:])
        Cb8s = wpool.tile([1, 2, 4, NH], FP8)
        nc.sync.dma_start(Cb8s[:], dCb8[:])
        ones8 = wpool.tile([1, 2, TN], FP8)
        nc.vector.memset(ones8[:], 1.0)

        cur = {}       # current supertile PSUM accumulators
        grp = {}       # current solve-group SBUF state
        pending = []   # deferred postproc/solve emission closures

        # ------------------------------------------------ forward pieces
        def act_block(a_psum, layer, sdt=F32):
            # s = sigmoid(-a); h-term = logsig(a) = ln(1 - s)
            sg = sb2.tile([128, 2, TN], sdt, tag=f"s{layer}", name=f"s{layer}")
            nc.scalar.activation(sg[:], a_psum[:], AF.Sigmoid, scale=-1.0)
            lg = sb2.tile([128, 2, TN], F32R, tag=f"lg{layer}", name=f"lg{layer}")
            nc.scalar.activation(lg[:], sg[:], AF.Ln, bias=1.0, scale=-1.0)
            return lg, sg

        def fwd_part1(t):
            e0 = t * TN
            zt = sb2.tile([NS, TN], F32R, tag="zt")
            nc.sync.dma_start(zt[:], zT[:, e0:e0 + TN])
            a_ps = psv_pool.tile([128, 2, TN], F32, tag="psv", name="a1ps")
            for j in range(2):
                nc.tensor.matmul(a_ps[:, j], L1s[:, j * 128:(j + 1) * 128],
                                 zt[:], start=True, stop=True)
            hm1, s1b = act_block(a_ps, 1)   # hm1 == h1 = logsig(a1)
            return dict(hm1=hm1, s1b=s1b)

        def fwd_part2(t, F):
            a_ps = psa_pool.tile([128, 2, TN], F32, tag="psa", name="a2ps")
            for j in range(2):
                for k in range(2):
                    nc.tensor.matmul(a_ps[:, j],
                                     L2s[:, k, j * 128:(j + 1) * 128],
                                     F["hm1"][:, k], start=(k == 0), stop=(k == 1))
            lg2, s2b = act_block(a_ps, 2)
            hm2 = sb2.tile([128, 2, TN], F32R, tag="hm2")
            nc.gpsimd.tensor_tensor(hm2[:], F["hm1"][:], lg2[:], OP.add)
            F.update(hm2=hm2, s2b=s2b)

        def fwd_part3(t, F):
            a_ps = psa_pool.tile([128, 2, TN], F32, tag="psa", name="a3ps")
            for j in range(2):
                for k in range(2):
                    nc.tensor.matmul(a_ps[:, j],
                                     L3s[:, k, j * 128:(j + 1) * 128],
                                     F["hm2"][:, k], start=(k == 0), stop=(k == 1))
            lg3, s3b = act_block(a_ps, 3, sdt=F32R)
            hm3 = sb2.tile([128, 2, TN], F32R, tag="hm3")
            nc.gpsimd.tensor_tensor(hm3[:], F["hm2"][:], lg3[:], OP.add)
            F.update(hm3=hm3, s3b=s3b)

        # ------------------------------------------------ backward of tile t
        # stage A(i): a*(V=C(I+D3W3)) -> v_ps[i]        (PE f32r + C fp8 DR)
        # stage B(i): v2s2 = KB*(V*s2)  fp8             (DVE stt)
        # stage C(i): v_ps[i] = a*U  (+= hi/lo fp8 DR)  (PE)
        # stage D(i): v1s1 = U*s1  bf16                 (DVE stt)
        # stage E(i): jt += J^T  bf16 smalls            (PE)
        SKEW = [(0, 0), (1, 0), (0, 1), (1, 1), (2, 0), (0, 2), (1, 2),
                (2, 1), (3, 0), (0, 3), (1, 3), (2, 2), (3, 1), (4, 0),
                (2, 3), (3, 2), (4, 1), (3, 3), (4, 2), (4, 3)]

        def make_backward(t, F):
            w = t % SUP          # position within supertile
            s = t // SUP         # supertile index
            sq = s % SOLVE_SUPS  # position within solve group
            last = (t == ntiles - 1)
            st = {}

            def start():
                ready = pending[:]
                pending.clear()
                st["ready"] = ready
                if w == 0:
                    cur["jt"] = tps_pool.tile([128, GRPS, 4 * NS], F32,
                                              tag="jtps", name="jt_ps")
                    cur["dt"] = tps_pool.tile([128, GRPS, NS], F32,
                                              tag="dtps", name="dt_ps")
                if not grp.get("open") and w == 0:
                    grp["open"] = True
                    grp["msb"] = mpool.tile([128, SROWS, 14], F32, tag="msb", name="msb")
                    grp["vsb"] = grp["msb"]
                    grp["csb"] = mpool.tile([128, SROWS, 4], F32, tag="csb", name="csb")
                    grp["jtg"] = jpool.tile([128, SROWS, 4 * NS], BF16, tag="jtg", name="jtg")
                    grp["dtg"] = jpool.tile([128, SROWS, NS], F32, tag="dtg", name="dtg")
                    grp["n"] = 0
                st["jt"], st["dt"] = cur["jt"], cur["dt"]

            def dt_emit():
                # d, transposed: stationary = hm3 chunk, moving = L4
                for g in range(4):
                    grpi = w * 4 + g
                    for k in range(2):
                        nc.tensor.matmul(st["dt"][:, grpi],
                                         F["hm3"][:, k, g * 128:(g + 1) * 128],
                                         L4s[:, k], start=(k == 0), stop=(k == 1))
            st["dt_emit"] = dt_emit

            v_tiles = [None] * 4
            v2s2_t = [None] * 4
            v1s1_t = [None] * 4

            def stage_a(i):
                v_ps = psv_pool.tile([128, 2, TN], F32, tag="psv", name=f"vps{i}")
                v_tiles[i] = v_ps
                for j in range(2):
                    for k in range(2):
                        nc.tensor.matmul(v_ps[:, j],
                                         M3s[:, k, i, j * 128:(j + 1) * 128],
                                         F["s3b"][:, k],
                                         start=(k == 0), stop=False)
                    nc.tensor.matmul(v_ps[:, j],
                                     Cb8s[:, :, i, j * 128:(j + 1) * 128],
                                     ones8[:], start=False, stop=False,
                                     perf_mode=DRM)

            def stage_b(i):
                v2s2 = sb.tile([128, 2, TN], FP8, tag="v2s2", name=f"v2s2_{i}")
                v2s2_t[i] = v2s2
                nc.vector.scalar_tensor_tensor(v2s2[:], v_tiles[i][:], KB / ALPHA,
                                               F["s2b"][:], OP.mult, OP.mult)

            def stage_c(i):
                v_ps = v_tiles[i]
                for j in range(2):
                    nc.tensor.matmul(v_ps[:, j], W2his[:, :, j * 128:(j + 1) * 128],
                                     v2s2_t[i][:], start=False, stop=False,
                                     perf_mode=DRM)
                    nc.tensor.matmul(v_ps[:, j], W2los[:, :, j * 128:(j + 1) * 128],
                                     v2s2_t[i][:], start=False, stop=True,
                                     perf_mode=DRM)

            def stage_d(i):
                v1s1 = sb.tile([128, 2, TN], BF16, tag="v1s1", name=f"v1s1_{i}")
                v1s1_t[i] = v1s1
                nc.vector.scalar_tensor_tensor(v1s1[:], v_tiles[i][:], 1.0 / ALPHA,
                                               F["s1b"][:], OP.mult, OP.mult)

            def stage_e(i):
                for g in range(4):
                    grpi = w * 4 + g
                    for k in range(2):
                        nc.tensor.matmul(st["jt"][:, grpi, i * NS:(i + 1) * NS],
                                         v1s1_t[i][:, k, g * 128:(g + 1) * 128],
                                         W1s[:, k], start=(k == 0), stop=(k == 1))

            stages = [stage_a, stage_b, stage_c, stage_d, stage_e]

            def chunk(lo, hi):
                for si, i in SKEW[lo:hi]:
                    stages[si](i)

            st["start"] = start
            st["chunk"] = chunk
            st["finish"] = lambda: finish_backward(t, w, s, sq, last, st)
            return st

        def finish_backward(t, w, s, sq, last, st):
            jt_ps, dt_ps = st["jt"], st["dt"]
            # -------- supertile done: drain PSUM, defer arithmetic
            if w == SUP - 1:
                qi = grp["n"]          # position within the current group
                rows = slice(qi * GRPS, (qi + 1) * GRPS)
                jtg, dtg = grp["jtg"], grp["dtg"]
                msb, vsb = grp["msb"], grp["vsb"]
                nc.scalar.copy(jtg[:, rows], jt_ps[:])
                nc.scalar.copy(dtg[:, rows], dt_ps[:])
                grp["n"] += 1

                def postproc(rows=rows, jtg=jtg, dtg=dtg, msb=msb, vsb=vsb):
                    jts = jtg[:, rows]
                    dts = dtg[:, rows]
                    # Gram products batched by index shift:
                    #   shift0 (0,0)(1,1)(2,2)(3,3) -> msb cols 0..3
                    #   shift1 (0,1)(1,2)(2,3)      -> cols 4..6
                    #   shift2 (0,2)(1,3)           -> cols 7..8
                    #   shift3 (0,3)                -> col  9
                    col0 = [0, 4, 7, 9]
                    prod = sb2b.tile([128, GRPS, 14, NS], BF16, tag="prod")
                    for sh in range(4):
                        na = 4 - sh
                        in0 = jts[:, :, 0:na * NS].rearrange(
                            "p g (a c) -> p g a c", a=na)
                        in1 = jts[:, :, sh * NS:(sh + na) * NS].rearrange(
                            "p g (a c) -> p g a c", a=na)
                        nc.gpsimd.tensor_tensor(
                            prod[:, :, col0[sh]:col0[sh] + na], in0, in1,
                            OP.mult)
                    in0 = jts[:, :, :].rearrange("p g (a c) -> p g a c", a=4)
                    in1 = dts[:, :, :].rearrange(
                        "p g (a c) -> p g a c", a=1).to_broadcast(
                            (128, GRPS, 4, NS))
                    nc.gpsimd.tensor_tensor(prod[:, :, 10:14], in0, in1, OP.mult)
                    nc.vector.tensor_reduce(msb[:, rows, 0:14], prod[:],
                                            AX.X, OP.add)

                pending.append(postproc)
                # close the group a supertile early at the end so the final
                # (tiny) group's serial solve tail is short
                if (grp["n"] == SOLVE_SUPS or last
                        or s == ntiles // SUP - 2):
                    grp["open"] = False
                    pending.extend(make_solve(grp["n"], s, grp["msb"], grp["vsb"],
                                              grp["csb"], grp["jtg"], grp["dtg"]))

            if st["ready"]:
                st["ready"].pop(0)()
                pending[0:0] = st["ready"]

        # ---------------- batched 4x4 solve + combine at end of solve group
        def make_solve(nsup, s, msb, vsb, csb, jtg, dtg):
            parts = []

            def part(f):
                parts.append(f)
                return f

            R = nsup * GRPS

            def m(i_):
                return msb[:, :R, i_:i_ + 1]

            def vv(i_):
                return vsb[:, :R, 10 + i_:11 + i_]

            tt = {}

            def tmp(name):
                if name not in tt:
                    tt[name] = mp1.tile([128, SROWS, 1], F32, tag=f"t_{name}", name=f"t_{name}")
                return tt[name][:, :R]

            V = nc.any

            def mul(o, x, y):
                V.tensor_tensor(o, x, y, OP.mult)

            def sub(o, x, y):
                V.tensor_tensor(o, x, y, OP.subtract)

            def add(o, x, y):
                V.tensor_tensor(o, x, y, OP.add)

            # index map: 0:00 1:11 2:22 3:33 4:01 5:12 6:23 7:02 8:13 9:03
            def p1():
                m00, m11, m01 = m(0), m(1), m(4)
                x1, x2 = tmp("x1"), tmp("x2")
                detA = tmp("detA")
                mul(x1, m00, m11); mul(x2, m01, m01); sub(detA, x1, x2)
                u10, u11 = tmp("u10"), tmp("u11")
                mul(x1, m11, vv(0)); mul(x2, m01, vv(1)); sub(u10, x1, x2)
                mul(x1, m00, vv(1)); mul(x2, m01, vv(0)); sub(u11, x1, x2)

            def p2():
                m00, m11, m01, m12, m02, m13, m03 = (
                    m(0), m(1), m(4), m(5), m(7), m(8), m(9))
                x1, x2 = tmp("x1"), tmp("x2")
                P00, P01, P10, P11 = tmp("P00"), tmp("P01"), tmp("P10"), tmp("P11")
                mul(x1, m11, m02); mul(x2, m01, m12); sub(P00, x1, x2)
                mul(x1, m11, m03); mul(x2, m01, m13); sub(P01, x1, x2)
                mul(x1, m00, m12); mul(x2, m01, m02); sub(P10, x1, x2)
                mul(x1, m00, m13); mul(x2, m01, m03); sub(P11, x1, x2)

            def p3():
                m22, m33, m12, m23, m02, m13, m03 = (
                    m(2), m(3), m(5), m(6), m(7), m(8), m(9))
                x1, x2 = tmp("x1"), tmp("x2")
                detA = tmp("detA")
                P00, P01, P10, P11 = tmp("P00"), tmp("P01"), tmp("P10"), tmp("P11")
                S00, S01, S11 = tmp("S00"), tmp("S01"), tmp("S11")
                mul(x1, m02, P00); mul(x2, m12, P10); add(x1, x1, x2)
                mul(S00, detA, m22); sub(S00, S00, x1)
                mul(x1, m02, P01); mul(x2, m12, P11); add(x1, x1, x2)
                mul(S01, detA, m23); sub(S01, S01, x1)
                mul(x1, m03, P01); mul(x2, m13, P11); add(x1, x1, x2)
                mul(S11, detA, m33); sub(S11, S11, x1)

            def p4():
                m12, m02, m13, m03 = m(5), m(7), m(8), m(9)
                x1, x2 = tmp("x1"), tmp("x2")
                detA, u10, u11 = tmp("detA"), tmp("u10"), tmp("u11")
                S00, S01, S11 = tmp("S00"), tmp("S01"), tmp("S11")
                w0, w1, detS = tmp("w0"), tmp("w1"), tmp("detS")
                mul(x1, m02, u10); mul(x2, m12, u11); add(x1, x1, x2)
                mul(w0, detA, vv(2)); sub(w0, w0, x1)
                mul(x1, m03, u10); mul(x2, m13, u11); add(x1, x1, x2)
                mul(w1, detA, vv(3)); sub(w1, w1, x1)
                mul(x1, S00, S11); mul(x2, S01, S01); sub(detS, x1, x2)

            def p5():
                x1, x2 = tmp("x1"), tmp("x2")
                u10, u11, detA = tmp("u10"), tmp("u11"), tmp("detA")
                S00, S01, S11 = tmp("S00"), tmp("S01"), tmp("S11")
                P00, P01, P10, P11 = tmp("P00"), tmp("P01"), tmp("P10"), tmp("P11")
                w0, w1, detS = tmp("w0"), tmp("w1"), tmp("detS")
                cw2, cw3 = tmp("cw2"), tmp("cw3")
                mul(x1, S11, w0); mul(x2, S01, w1); sub(cw2, x1, x2)
                mul(x1, S00, w1); mul(x2, S01, w0); sub(cw3, x1, x2)
                q0, q1 = tmp("q0"), tmp("q1")
                mul(x1, P00, cw2); mul(x2, P01, cw3); add(x1, x1, x2)
                mul(q0, u10, detS); sub(q0, q0, x1)
                mul(x1, P10, cw2); mul(x2, P11, cw3); add(x1, x1, x2)
                mul(q1, u11, detS); sub(q1, q1, x1)

            def p6():
                detA, detS = tmp("detA"), tmp("detS")
                q0, q1, cw2, cw3 = tmp("q0"), tmp("q1"), tmp("cw2"), tmp("cw3")
                dAS, rAS, rS = tmp("dAS"), tmp("rAS"), tmp("rS")
                mul(dAS, detA, detS)
                nc.vector.reciprocal(rAS, dAS)
                nc.vector.reciprocal(rS, detS)
                mul(csb[:, :R, 0:1], q0, rAS)
                mul(csb[:, :R, 1:2], q1, rAS)
                mul(csb[:, :R, 2:3], cw2, rS)
                mul(csb[:, :R, 3:4], cw3, rS)

            # combine + write out
            s_base = s - (nsup - 1)
            R2 = nsup * GRPS
            cst = {}

            def comb(a0, a1):
                def f():
                    G = nc.gpsimd
                    if "acc" not in cst:
                        cst["acc"] = sb2b.tile([128, SROWS, NS], F32, tag="acc",
                                               name="acc")
                        cst["ctmp"] = sb2b.tile([128, SROWS, NS], F32,
                                                tag="ctmp", name="ctmp")
                    acc, ctmp = cst["acc"], cst["ctmp"]
                    for a in range(a0, a1):
                        cb = csb[:, :R2, a:a + 1].to_broadcast((128, R2, NS))
                        G.tensor_tensor(ctmp[:, :R2], cb,
                                        jtg[:, :R2, a * NS:(a + 1) * NS],
                                        OP.mult)
                        if a == 0:
                            G.tensor_tensor(acc[:, :R2], dtg[:, :R2],
                                            ctmp[:, :R2], OP.subtract)
                        else:
                            G.tensor_tensor(acc[:, :R2], acc[:, :R2],
                                            ctmp[:, :R2], OP.subtract)
                return f

            def dma_out():
                eb = s_base * SUP * TN
                nc.sync.dma_start(
                    out_d[eb:eb + R2 * 128, :].rearrange(
                        "(g p) m -> p g m", p=128),
                    cst["acc"][:, :R2])

            return [p1, p2, p3, p4, p5, p6, comb(0, 2), comb(2, 4), dma_out]

        # ------------------------------------------------ main loop
        # forward(t+1) is emitted before backward(t): the in-order PE stream
        # gets next-tile matmul work to chew on while tile t's activation
        # chain completes, and vice versa.
        F = None
        for t in range(ntiles + 1):
            Fn = fwd_part1(t) if t < ntiles else None
            B = make_backward(t - 1, F) if F is not None else None
            if B:
                B["start"]()
                B["chunk"](0, 4)          # A0,B0,A1,B1
            if Fn:
                fwd_part2(t, Fn)
            if B:
                B["chunk"](4, 13)         # C0,A2,B2,C1,D0,A3,B3,C2,D1
                B["dt_emit"]()
            if Fn:
                fwd_part3(t, Fn)
            if B:
                B["chunk"](13, 20)        # E0,C3,D2,E1,D3,E2,E3
                B["finish"]()
            F = Fn
        for c in pending:
            c()

    _split_multi_waits(nc)
    return nc


# ---------------------------------------------------------------- entry point
def kernel(zstates, W1, W2, W3, W4):
    from concourse.bass_utils import run_bass_kernel_spmd

    key = "full"
    if key not in _cache:
        _cache[key] = _build()
    nc = _cache[key]

    wm = _prep_weights(W1, W2, W3, W4)
    z = np.asarray(zstates, np.float32).reshape(NCORES, BP, NS)
    in_maps = [
        {**wm, "zT": np.ascontiguousarray(z[c].T)} for c in range(NCORES)
    ]
    res = run_bass_kernel_spmd(nc, in_maps, core_ids=list(range(NCORES)))
    return np.concatenate([res.results[c]["out"] for c in range(NCORES)], axis=0)

